# revision 9
# baseline (speedup 1.0000x reference)
"""Trainium2 Bass kernel for nn_Encoder_14018773254741 (2-layer SAGE-GCN
encoder + soft-target CE decoder) on 8 NeuronCores.

Strategy (1D node partition):
  - 40000 nodes split 5000/core; each core owns the edges whose dst lands in
    its range (plus one explicit self-edge per node, which implements the
    "+ h_v" term of the gcn aggregator).
  - Edges are sorted by dst and packed into fixed-size tiles of 128 on the
    host; per 125-node block the aggregation runs as: batched indirect-DMA
    gather of h[src] rows (bf16 table) -> one-hot selection matrix built
    on-chip (iota == dst_local) -> PE matmuls accumulating neigh^T in PSUM.
  - The 128-dim linears replicate weights; deg-scaling is folded in after
    the FC ((S*A) @ W == S*(A @ W)); dropout masks are precomputed constants
    (fixed jax key) applied on-chip.
  - Halo exchange: the full (quantized bf16) node-feature table is
    AllGather-ed between layers; decoder logits z are AllGather-ed once for
    the shuffled-row gather.

Outputs: h [40000,128] f32 (node-major shards concatenated on host) and the
scalar dec (per-core partial sums, final mean on host).
"""

import numpy as np
import ml_dtypes

# ---- hardcoded problem config ------------------------------------------------
N = 40000          # nodes
D = 128            # feature dim (in = hid = out)
DEC = 64           # decoder dim
NC = 8             # cores
NP = N // NC       # nodes per core
BLK = 125          # dst nodes per aggregation block
NB = NP // BLK     # blocks per core
P = 128            # partitions / edge-tile size

_FULL_CFG = dict(N=N, D=D, DEC=DEC, NC=NC, NP=NP, BLK=BLK, NB=NB, P=P)

_program_cache = {}


# ---- device program ----------------------------------------------------------

def build_program(cfg, T, NTI, debug=False):
    """Build the 8-core SPMD Bass program. T = edge tiles per block,
    NTI = decoder gather tiles (= ceil(NP/128))."""
    import concourse.bass as bass
    import concourse.bacc as bacc
    import concourse.mybir as mybir
    import concourse.tile as tile
    from concourse.masks import make_identity

    f32 = mybir.dt.float32
    bf16 = mybir.dt.bfloat16
    i32 = mybir.dt.int32
    Alu = mybir.AluOpType
    Act = mybir.ActivationFunctionType

    cN, cD, cDEC, cNC, cNP, cBLK, cNB, cP = (
        cfg["N"], cfg["D"], cfg["DEC"], cfg["NC"], cfg["NP"], cfg["BLK"],
        cfg["NB"], cfg["P"])
    AG = [list(range(cNC))]

    nc = bacc.Bacc("TRN2", target_bir_lowering=False, debug=False,
                   num_devices=cNC)

    def din(name, shape, dt):
        return nc.dram_tensor(name, shape, dt, kind="ExternalInput").ap()

    x_own = din("x_own", [cNP, cD], bf16)
    m1_own = din("m1_own", [cNP, cD], bf16)
    m2_own = din("m2_own", [cNP, cD], bf16)
    src_e = din("src_e", [cP, cNB * T], i32)
    dst_e = din("dst_e", [cP, cNB * T], f32)
    scl_own = din("scl_own", [cBLK, cNB], f32)
    shuf = din("shuf", [cP, NTI], i32)
    w1t = din("w1t", [cD, cD], bf16)
    w2t = din("w2t", [cD, cD], bf16)
    wdt = din("wdt", [cD, cDEC], bf16)
    b1bc = din("b1bc", [cP, cD], f32)
    b2bc = din("b2bc", [cP, cD], f32)
    bdbc = din("bdbc", [cP, cDEC], f32)
    iota_in = din("iota", [cP, cBLK], f32)

    h2_out = nc.dram_tensor("h2_out", [cNP, cD], f32,
                            kind="ExternalOutput").ap()
    dec_out = nc.dram_tensor("dec_out", [1, 1], f32,
                             kind="ExternalOutput").ap()
    if debug:
        dbg_h0 = nc.dram_tensor("dbg_h0", [cNP, cD], f32,
                                kind="ExternalOutput").ap()
        dbg_msg = nc.dram_tensor("dbg_msg", [cP, T * cD], f32,
                                 kind="ExternalOutput").ap()
        dbg_mp = nc.dram_tensor("dbg_mp", [cP, T * cBLK], f32,
                                kind="ExternalOutput").ap()
        dbg_nT = nc.dram_tensor("dbg_nT", [cD, cBLK], f32,
                                kind="ExternalOutput").ap()
        dbg_h1 = nc.dram_tensor("dbg_h1", [cNP, cD], f32,
                                kind="ExternalOutput").ap()
        dbg_z = nc.dram_tensor("dbg_z", [cNP, cDEC], f32,
                               kind="ExternalOutput").ap()
        dbg_ls = nc.dram_tensor("dbg_ls", [cNP, cDEC], f32,
                                kind="ExternalOutput").ap()

    with tile.TileContext(nc) as tc:
        with (
            tc.tile_pool(name="cst", bufs=1) as cst,
            tc.tile_pool(name="big", bufs=1) as big,
            tc.tile_pool(name="sb", bufs=2) as sb,
            tc.tile_pool(name="psn", bufs=2, space="PSUM") as psn,
            tc.tile_pool(name="psf", bufs=2, space="PSUM") as psf,
            tc.tile_pool(name="pst", bufs=1, space="PSUM") as pst,
            tc.tile_pool(name="psz", bufs=1, space="PSUM") as psz,
            tc.tile_pool(name="dram", bufs=1, space="DRAM") as dram,
        ):
            # ---- persistent constants ----
            srcs = cst.tile([cP, cNB * T], i32)
            nc.sync.dma_start(out=srcs[:], in_=src_e[:, :])
            dsts = cst.tile([cP, cNB * T], f32)
            nc.sync.dma_start(out=dsts[:], in_=dst_e[:, :])
            iot = cst.tile([cP, cBLK], f32)
            nc.sync.dma_start(out=iot[:], in_=iota_in[:, :])
            sclc = cst.tile([cBLK, cNB], f32)
            nc.sync.dma_start(out=sclc[:], in_=scl_own[:, :])
            w1 = cst.tile([cD, cD], bf16)
            nc.sync.dma_start(out=w1[:], in_=w1t[:, :])
            w2 = cst.tile([cD, cD], bf16)
            nc.sync.dma_start(out=w2[:], in_=w2t[:, :])
            wd = cst.tile([cD, cDEC], bf16)
            nc.sync.dma_start(out=wd[:], in_=wdt[:, :])
            bb1 = cst.tile([cP, cD], f32)
            nc.sync.dma_start(out=bb1[:], in_=b1bc[:, :])
            bb2 = cst.tile([cP, cD], f32)
            nc.sync.dma_start(out=bb2[:], in_=b2bc[:, :])
            bbd = cst.tile([cP, cDEC], f32)
            nc.sync.dma_start(out=bbd[:], in_=bdbc[:, :])
            shufs = cst.tile([cP, NTI], i32)
            nc.sync.dma_start(out=shufs[:], in_=shuf[:, :])
            ident = cst.tile([cP, cP], f32)
            make_identity(nc, ident[:])
            m2res = cst.tile([cBLK, cNB * cD], bf16)
            nc.sync.dma_start(
                out=m2res[:].rearrange("n (b d) -> n b d", b=cNB),
                in_=m2_own.rearrange("(b n) d -> n b d", n=cBLK))

            # persistent per-layer result accumulators
            h1res = cst.tile([cBLK, cNB * cD], bf16)
            h2res = cst.tile([cBLK, cNB * cD], f32)
            zres = cst.tile([cBLK, cNB * cDEC], bf16)
            lsres = cst.tile([cBLK, cNB * cDEC], f32)

            # ---- DRAM intermediates ----
            h0b = dram.tile([cNP, cD], bf16)
            h0f = dram.tile([cN, cD], bf16)
            h1b = dram.tile([cNP, cD], bf16)
            h1f = dram.tile([cN, cD], bf16)
            zb = dram.tile([cNP, cDEC], bf16)
            zf = dram.tile([cN, cDEC], bf16)
            lsd = dram.tile([NTI * cP, cDEC], f32)

            # ---- phase A: h0 = x * mask1 (bf16 table) ----
            xres = big.tile([cBLK, cNB * cD], bf16, tag="big20")
            nc.sync.dma_start(
                out=xres[:].rearrange("n (b d) -> n b d", b=cNB),
                in_=x_own.rearrange("(b n) d -> n b d", n=cBLK))
            m1res = big.tile([cBLK, cNB * cD], bf16, tag="bigA")
            nc.sync.dma_start(
                out=m1res[:].rearrange("n (b d) -> n b d", b=cNB),
                in_=m1_own.rearrange("(b n) d -> n b d", n=cBLK))
            h0res = big.tile([cBLK, cNB * cD], bf16, tag="bigB")
            nc.vector.tensor_tensor(out=h0res[:], in0=xres[:], in1=m1res[:],
                                    op=Alu.mult)
            nc.sync.dma_start(
                out=h0b[:].rearrange("(b n) d -> n b d", n=cBLK),
                in_=h0res[:].rearrange("n (b d) -> n b d", b=cNB))
            if debug:
                nc.gpsimd.dma_start(
                    out=dbg_h0.rearrange("(b n) d -> n b d", n=cBLK),
                    in_=h0res[:].rearrange("n (b d) -> n b d", b=cNB))
            nc.gpsimd.collective_compute(
                "AllGather", Alu.bypass, replica_groups=AG,
                ins=[h0b[:].opt()], outs=[h0f[:].opt()])

            # ---- shared aggregation + FC ----
            def emit_layer(table, wt, layer_idx):
                for b in range(cNB):
                    # per-tile [128,1] gathers of this block's edge sources
                    # (the only indirect-DMA form the HW handles correctly)
                    msg = sb.tile([cP, T * cD], bf16, tag="msg")
                    msgv0 = msg[:].rearrange("p (t d) -> p t d", t=T)
                    for t in range(T):
                        nc.gpsimd.indirect_dma_start(
                            out=msgv0[:, t, :],
                            out_offset=None,
                            in_=table[:],
                            in_offset=bass.IndirectOffsetOnAxis(
                                ap=srcs[:, b * T + t:b * T + t + 1], axis=0),
                        )
                    # selection matrix M[e, t, s] = (dst_local[e,t] == s)
                    mp = sb.tile([cP, T * cBLK], bf16, tag="mp")
                    dstb = dsts[:, b * T:(b + 1) * T].to_broadcast(
                        [cP, T, cBLK])
                    iob = iot[:].rearrange("p (o k) -> p o k", o=1)\
                        .to_broadcast([cP, T, cBLK])
                    nc.vector.tensor_tensor(
                        out=mp[:].rearrange("p (t k) -> p t k", t=T),
                        in0=dstb, in1=iob, op=Alu.is_equal)
                    # neigh^T[f, s] accumulated over edge tiles
                    pn = psn.tile([cD, cBLK], mybir.dt.float32, tag="pn")
                    msgv = msg[:].rearrange("p (t d) -> p t d", t=T)
                    mpv = mp[:].rearrange("p (t k) -> p t k", t=T)
                    for t in range(T):
                        nc.tensor.matmul(
                            pn[:], lhsT=msgv[:, t, :], rhs=mpv[:, t, :],
                            start=(t == 0), stop=(t == T - 1))
                    nT = sb.tile([cD, cBLK], bf16, tag="nT")
                    nc.vector.tensor_copy(out=nT[:], in_=pn[:])
                    if debug and layer_idx == 1 and b == 0:
                        nc.gpsimd.dma_start(out=dbg_msg[:, :], in_=msg[:])
                        nc.gpsimd.dma_start(out=dbg_mp[:, :], in_=mp[:])
                        nc.gpsimd.dma_start(out=dbg_nT[:, :], in_=nT[:])
                    # FC: out[n, o] = neigh^T.T @ W^T  (scale folded after)
                    pf = psf.tile([cBLK, cD], mybir.dt.float32, tag="pf")
                    nc.tensor.matmul(pf[:], lhsT=nT[:], rhs=wt[:],
                                     start=True, stop=True)
                    t1 = sb.tile([cBLK, cD], f32, tag="t1")
                    nc.vector.tensor_scalar(
                        out=t1[:], in0=pf[:], scalar1=sclc[:, b:b + 1],
                        scalar2=None, op0=Alu.mult)
                    if layer_idx == 1:
                        t2 = sb.tile([cBLK, cD], f32, tag="t2")
                        nc.vector.tensor_tensor(out=t2[:], in0=t1[:],
                                                in1=bb1[:cBLK, :], op=Alu.add)
                        # relu(x)*m == relu(x*m) since m >= 0
                        t3 = sb.tile([cBLK, cD], f32, tag="t3")
                        m2v = m2res[:].rearrange("n (b d) -> n b d", b=cNB)
                        nc.vector.tensor_tensor(out=t3[:], in0=t2[:],
                                                in1=m2v[:, b, :], op=Alu.mult)
                        h1v = h1res[:].rearrange("n (b d) -> n b d", b=cNB)
                        nc.vector.tensor_scalar(
                            out=h1v[:, b, :], in0=t3[:], scalar1=0.0,
                            scalar2=None, op0=Alu.max)
                    else:
                        h2v = h2res[:].rearrange("n (b d) -> n b d", b=cNB)
                        nc.vector.tensor_tensor(out=h2v[:, b, :], in0=t1[:],
                                                in1=bb2[:cBLK, :], op=Alu.add)
                        # transpose h2 block for the decoder FC
                        pt = pst.tile([cD, cBLK], mybir.dt.float32, tag="pt")
                        nc.tensor.transpose(pt[:], h2v[:, b, :],
                                            ident[:cBLK, :cBLK])
                        hT = sb.tile([cD, cBLK], bf16, tag="hT")
                        nc.vector.tensor_copy(out=hT[:], in_=pt[:])
                        pz = psz.tile([cBLK, cDEC], mybir.dt.float32,
                                      tag="pz")
                        nc.tensor.matmul(pz[:], lhsT=hT[:], rhs=wd[:],
                                         start=True, stop=True)
                        zsb = sb.tile([cBLK, cDEC], f32, tag="zsb")
                        nc.vector.tensor_tensor(out=zsb[:], in0=pz[:],
                                                in1=bbd[:cBLK, :], op=Alu.add)
                        zv = zres[:].rearrange("n (b d) -> n b d", b=cNB)
                        nc.vector.tensor_copy(out=zv[:, b, :], in_=zsb[:])
                        # log-softmax over DEC
                        negm = sb.tile([cBLK, 1], f32, tag="negm")
                        nc.vector.tensor_reduce(
                            out=negm[:], in_=zsb[:],
                            axis=mybir.AxisListType.X, op=Alu.max,
                            negate=True)
                        esb = sb.tile([cBLK, cDEC], f32, tag="esb")
                        nc.scalar.activation(esb[:], zsb[:], Act.Exp,
                                             bias=negm[:, 0:1], scale=1.0)
                        ssum = sb.tile([cBLK, 1], f32, tag="ssum")
                        nc.vector.tensor_reduce(
                            out=ssum[:], in_=esb[:],
                            axis=mybir.AxisListType.X, op=Alu.add)
                        lns = sb.tile([cBLK, 1], f32, tag="lns")
                        nc.scalar.activation(lns[:], ssum[:], Act.Ln)
                        mpl = sb.tile([cBLK, 1], f32, tag="mpl")
                        nc.vector.tensor_tensor(out=mpl[:], in0=lns[:],
                                                in1=negm[:], op=Alu.subtract)
                        lsv = lsres[:].rearrange("n (b d) -> n b d", b=cNB)
                        nc.vector.tensor_scalar(
                            out=lsv[:, b, :], in0=zsb[:], scalar1=mpl[:, 0:1],
                            scalar2=None, op0=Alu.subtract)

            # ---- layer 1 ----
            emit_layer(h0f, w1, 1)
            nc.sync.dma_start(
                out=h1b[:].rearrange("(b n) d -> n b d", n=cBLK),
                in_=h1res[:].rearrange("n (b d) -> n b d", b=cNB))
            if debug:
                nc.gpsimd.dma_start(
                    out=dbg_h1.rearrange("(b n) d -> n b d", n=cBLK),
                    in_=h1res[:].rearrange("n (b d) -> n b d", b=cNB))
            nc.gpsimd.collective_compute(
                "AllGather", Alu.bypass, replica_groups=AG,
                ins=[h1b[:].opt()], outs=[h1f[:].opt()])

            # ---- layer 2 + decoder per-block ----
            emit_layer(h1f, w2, 2)
            nc.sync.dma_start(
                out=h2_out.rearrange("(b n) d -> n b d", n=cBLK),
                in_=h2res[:].rearrange("n (b d) -> n b d", b=cNB))
            nc.sync.dma_start(
                out=zb[:].rearrange("(b n) d -> n b d", n=cBLK),
                in_=zres[:].rearrange("n (b d) -> n b d", b=cNB))
            nc.gpsimd.collective_compute(
                "AllGather", Alu.bypass, replica_groups=AG,
                ins=[zb[:].opt()], outs=[zf[:].opt()])
            if debug:
                nc.gpsimd.dma_start(
                    out=dbg_z.rearrange("(b n) d -> n b d", n=cBLK),
                    in_=zres[:].rearrange("n (b d) -> n b d", b=cNB))
                nc.sync.dma_start(
                    out=dbg_ls.rearrange("(b n) d -> n b d", n=cBLK),
                    in_=lsres[:].rearrange("n (b d) -> n b d", b=cNB))
            nc.sync.dma_start(
                out=lsd[0:cNP, :].rearrange("(b n) d -> n b d", n=cBLK),
                in_=lsres[:].rearrange("n (b d) -> n b d", b=cNB))
            pad_rows = NTI * cP - cNP
            if pad_rows:
                zpad = sb.tile([pad_rows, cDEC], f32, tag="zpad")
                nc.vector.memset(zpad[:], 0.0)
                nc.sync.dma_start(out=lsd[cNP:, :], in_=zpad[:])

            # ---- decoder tail ----
            lst = big.tile([cP, NTI * cDEC], f32, tag="bigA")
            nc.sync.dma_start(
                out=lst[:].rearrange("p (i d) -> p i d", i=NTI),
                in_=lsd[:].rearrange("(i p) d -> p i d", p=cP))
            zg = big.tile([cP, NTI * cDEC], bf16, tag="bigB")
            zgv = zg[:].rearrange("p (i d) -> p i d", i=NTI)
            for i in range(NTI):
                nc.gpsimd.indirect_dma_start(
                    out=zgv[:, i, :],
                    out_offset=None,
                    in_=zf[:],
                    in_offset=bass.IndirectOffsetOnAxis(
                        ap=shufs[:, i:i + 1], axis=0),
                )
            pr = big.tile([cP, NTI * cDEC], f32, tag="big20")
            nc.vector.tensor_tensor(out=pr[:], in0=zg[:], in1=lst[:],
                                    op=Alu.mult)
            ac = sb.tile([cP, 1], f32, tag="ac")
            nc.vector.tensor_reduce(
                out=ac[:], in_=pr[:].rearrange("p (i d) -> p i d", i=NTI),
                axis=mybir.AxisListType.XY, op=Alu.add)
            ones = cst.tile([cP, 1], f32)
            nc.vector.memset(ones[:], 1.0)
            pd = psz.tile([1, 1], mybir.dt.float32, tag="pd")
            nc.tensor.matmul(pd[:], lhsT=ac[:], rhs=ones[:], start=True,
                             stop=True)
            dsb = sb.tile([1, 1], f32, tag="dsb")
            nc.vector.tensor_copy(out=dsb[:], in_=pd[:])
            nc.sync.dma_start(out=dec_out[:, :], in_=dsb[:])

    nc.compile()
    return nc


# ---- host-side sharding / preprocessing --------------------------------------

def host_prep(cfg, x, src, dst, shuffled_index, W1, b1, W2, b2, Wd, bd,
              mask1, mask2):
    """Partition the graph and pack per-core DMA-friendly arrays.
    Returns (in_maps, T, NTI)."""
    cN, cD, cDEC, cNC, cNP, cBLK, cNB, cP = (
        cfg["N"], cfg["D"], cfg["DEC"], cfg["NC"], cfg["NP"], cfg["BLK"],
        cfg["NB"], cfg["P"])
    bf = ml_dtypes.bfloat16

    src = np.asarray(src).astype(np.int64)
    dst = np.asarray(dst).astype(np.int64)
    shuffled_index = np.asarray(shuffled_index).astype(np.int64)
    x = np.ascontiguousarray(np.asarray(x, dtype=np.float32))

    deg = np.bincount(dst, minlength=cN)
    scale = (1.0 / (deg + 1.0)).astype(np.float32)

    # sort edges by dst; blocks never straddle cores since NP % BLK == 0
    order = np.argsort(dst, kind="stable")
    s_src = src[order]
    s_dst = dst[order]
    gblk = s_dst // cBLK                      # global block id, 0..NC*NB-1
    nblk = cNC * cNB
    counts = np.bincount(gblk, minlength=nblk)
    ptr = np.zeros(nblk + 1, np.int64)
    np.cumsum(counts, out=ptr[1:])
    tot = counts + cBLK                       # + self edges
    T = int(np.ceil(tot.max() / cP))
    slots = T * cP

    src_all = np.zeros(nblk * slots, np.int32)          # pad -> row 0
    dst_all = np.full(nblk * slots, -1.0, np.float32)   # pad -> no slot
    # real edges
    rank = np.arange(len(s_src)) - ptr[gblk]
    pos = gblk * slots + rank
    src_all[pos] = s_src
    dst_all[pos] = (s_dst % cBLK).astype(np.float32)
    # self edges
    g = np.arange(nblk)
    vbase = (g // cNB) * cNP + (g % cNB) * cBLK
    selfpos = (g[:, None] * slots + counts[:, None]
               + np.arange(cBLK)[None, :]).ravel()
    src_all[selfpos] = (vbase[:, None] + np.arange(cBLK)[None, :]).ravel()
    dst_all[selfpos] = np.tile(np.arange(cBLK, dtype=np.float32), nblk)

    # [core, (b t p)] -> [core][P, NB*T] with col = b*T+t, lane p
    src_all = src_all.reshape(cNC, cNB * T, cP)
    dst_all = dst_all.reshape(cNC, cNB * T, cP)

    NTI = -(-cNP // cP)
    shuf_pad = np.zeros(cNC * NTI * cP, np.int32)
    shuf_pad[: cNC * cNP] = shuffled_index.reshape(cNC, cNP).ravel()
    # careful: padding must be per core
    shuf_pc = np.zeros((cNC, NTI * cP), np.int32)
    shuf_pc[:, :cNP] = shuffled_index.reshape(cNC, cNP)
    shuf_pc = shuf_pc.reshape(cNC, NTI, cP)

    scale_pc = scale.reshape(cNC, cNB, cBLK)

    w1t = np.ascontiguousarray(np.asarray(W1, np.float32).T).astype(bf)
    w2t = np.ascontiguousarray(np.asarray(W2, np.float32).T).astype(bf)
    wdt = np.ascontiguousarray(np.asarray(Wd, np.float32).T).astype(bf)
    b1bc = np.ascontiguousarray(
        np.broadcast_to(np.asarray(b1, np.float32), (cP, cD)))
    b2bc = np.ascontiguousarray(
        np.broadcast_to(np.asarray(b2, np.float32), (cP, cD)))
    bdbc = np.ascontiguousarray(
        np.broadcast_to(np.asarray(bd, np.float32), (cP, cDEC)))
    iota = np.ascontiguousarray(
        np.broadcast_to(np.arange(cBLK, dtype=np.float32), (cP, cBLK)))

    m1 = np.asarray(mask1, np.float32).astype(bf).reshape(cNC, cNP, cD)
    xbf = x.astype(bf)
    m2 = np.asarray(mask2, np.float32).astype(bf).reshape(cNC, cNP, cD)
    xs = xbf.reshape(cNC, cNP, cD)

    in_maps = []
    for c in range(cNC):
        in_maps.append({
            "x_own": np.ascontiguousarray(xs[c]),
            "m1_own": np.ascontiguousarray(m1[c]),
            "m2_own": np.ascontiguousarray(m2[c]),
            "src_e": np.ascontiguousarray(src_all[c].T),
            "dst_e": np.ascontiguousarray(dst_all[c].T),
            "scl_own": np.ascontiguousarray(scale_pc[c].T),
            "shuf": np.ascontiguousarray(
                shuf_pc[c].T.reshape(cP, NTI)),
            "w1t": w1t, "w2t": w2t, "wdt": wdt,
            "b1bc": b1bc, "b2bc": b2bc, "bdbc": bdbc,
            "iota": iota,
        })
    return in_maps, T, NTI


# ---- execution ---------------------------------------------------------------

class PjrtRunner:
    """Mirror of bass2jax.run_bass_via_pjrt that keeps the jitted callable
    so repeat executions skip retracing."""

    def __init__(self, nc, n_cores):
        import jax
        import numpy as np
        from jax.sharding import Mesh, PartitionSpec
        from jax.experimental.shard_map import shard_map
        import concourse.mybir as mybir
        from concourse import bass2jax

        bass2jax.install_neuronx_cc_hook()
        self.jax = jax
        self.nc = nc
        self.n_cores = n_cores
        partition_name = (nc.partition_id_tensor.name
                          if nc.partition_id_tensor else None)
        in_names, out_names, out_avals, zero_outs = [], [], [], []
        for alloc in nc.m.functions[0].allocations:
            if not isinstance(alloc, mybir.MemoryLocationSet):
                continue
            name = alloc.memorylocations[0].name
            if alloc.kind == "ExternalInput":
                if name != partition_name:
                    in_names.append(name)
            elif alloc.kind == "ExternalOutput":
                shape = tuple(alloc.tensor_shape)
                dtype = mybir.dt.np(alloc.dtype)
                out_names.append(name)
                out_avals.append(jax.core.ShapedArray(shape, dtype))
                zero_outs.append(np.zeros(shape, dtype))
        self.in_names = in_names
        self.out_names = out_names
        self.out_avals = out_avals
        self.zero_outs = zero_outs
        n_params = len(in_names)
        n_outs = len(out_avals)
        all_names = list(in_names) + list(out_names)
        if partition_name is not None:
            all_names.append(partition_name)

        def _body(*args):
            operands = list(args)
            if partition_name is not None:
                operands.append(bass2jax.partition_id_tensor())
            outs = bass2jax._bass_exec_p.bind(
                *operands, out_avals=tuple(out_avals),
                in_names=tuple(all_names), out_names=tuple(out_names),
                lowering_input_output_aliases=(),
                sim_require_finite=True, sim_require_nnan=True, nc=nc)
            return tuple(outs)

        devices = jax.devices()[:n_cores]
        mesh = Mesh(np.asarray(devices), ("core",))
        in_specs = (PartitionSpec("core"),) * (n_params + n_outs)
        out_specs = (PartitionSpec("core"),) * n_outs
        self.sharded = jax.jit(
            shard_map(_body, mesh=mesh, in_specs=in_specs,
                      out_specs=out_specs, check_rep=False),
            keep_unused=True)
        self._dev_args = None

    def put(self, in_maps):
        np_ = np
        concat_in = [
            np_.concatenate([np_.asarray(in_maps[c][nm])
                             for c in range(self.n_cores)], axis=0)
            for nm in self.in_names]
        concat_zeros = [
            np_.zeros((self.n_cores * z.shape[0], *z.shape[1:]), z.dtype)
            for z in self.zero_outs]
        self._dev_args = [self.jax.device_put(a)
                          for a in concat_in + concat_zeros]

    def run(self):
        outs = self.sharded(*self._dev_args)
        self.jax.block_until_ready(outs)
        return [
            {nm: np.asarray(outs[i]).reshape(
                self.n_cores, *self.out_avals[i].shape)[c]
             for i, nm in enumerate(self.out_names)}
            for c in range(self.n_cores)]


def _compute_masks(shape1, shape2):
    import jax
    cpu = jax.devices("cpu")[0]
    with jax.default_device(cpu):
        dk = jax.random.split(jax.random.key(42), 2)
        m1 = np.asarray(
            jax.random.bernoulli(dk[0], 0.5, shape1)).astype(np.float32) * 2.0
        m2 = np.asarray(
            jax.random.bernoulli(dk[1], 0.5, shape2)).astype(np.float32) * 2.0
    return m1, m2


def kernel(x, src, dst, shuffled_index, W1, b1, W2, b2, Wd, bd):
    cfg = _FULL_CFG
    mask1, mask2 = _compute_masks((cfg["N"], cfg["D"]), (cfg["N"], cfg["D"]))
    in_maps, T, NTI = host_prep(cfg, x, src, dst, shuffled_index,
                                W1, b1, W2, b2, Wd, bd, mask1, mask2)
    key = (T, NTI)
    if key not in _program_cache:
        nc = build_program(cfg, T, NTI)
        _program_cache[key] = PjrtRunner(nc, cfg["NC"])
    runner = _program_cache[key]
    runner.put(in_maps)
    res = runner.run()
    h = np.concatenate([res[c]["h2_out"] for c in range(cfg["NC"])], axis=0)
    dec = -sum(float(res[c]["dec_out"][0, 0]) for c in range(cfg["NC"])) \
        / cfg["N"]
    return h.astype(np.float32), np.float32(dec)


# revision 10
# speedup vs baseline: 1.3626x; 1.3626x over previous
"""Trainium2 Bass kernel for nn_Encoder_14018773254741 (2-layer SAGE-GCN
encoder + soft-target CE decoder) on 8 NeuronCores.

Strategy (1D node partition):
  - 40000 nodes split 5000/core; each core owns the edges whose dst lands in
    its range (plus one explicit self-edge per node, which implements the
    "+ h_v" term of the gcn aggregator).
  - Edges are sorted by dst and packed into fixed-size tiles of 128 on the
    host; per 125-node block the aggregation runs as: batched indirect-DMA
    gather of h[src] rows (bf16 table) -> one-hot selection matrix built
    on-chip (iota == dst_local) -> PE matmuls accumulating neigh^T in PSUM.
  - The 128-dim linears replicate weights; deg-scaling is folded in after
    the FC ((S*A) @ W == S*(A @ W)); dropout masks are precomputed constants
    (fixed jax key) applied on-chip.
  - Halo exchange: the full (quantized bf16) node-feature table is
    AllGather-ed between layers; decoder logits z are AllGather-ed once for
    the shuffled-row gather.

Outputs: h [40000,128] f32 (node-major shards concatenated on host) and the
scalar dec (per-core partial sums, final mean on host).
"""

import numpy as np
import ml_dtypes

# ---- hardcoded problem config ------------------------------------------------
N = 40000          # nodes
D = 128            # feature dim (in = hid = out)
DEC = 64           # decoder dim
NC = 8             # cores
NP = N // NC       # nodes per core
BLK = 125          # dst nodes per aggregation block
NB = NP // BLK     # blocks per core
P = 128            # partitions / edge-tile size

_FULL_CFG = dict(N=N, D=D, DEC=DEC, NC=NC, NP=NP, BLK=BLK, NB=NB, P=P)

_program_cache = {}


# ---- device program ----------------------------------------------------------

def build_program(cfg, T, NTI, debug=False):
    """Build the 8-core SPMD Bass program. T = edge tiles per block,
    NTI = decoder gather tiles (= ceil(NP/128))."""
    import concourse.bass as bass
    import concourse.bacc as bacc
    import concourse.mybir as mybir
    import concourse.tile as tile
    from concourse.masks import make_identity

    f32 = mybir.dt.float32
    bf16 = mybir.dt.bfloat16
    i32 = mybir.dt.int32
    Alu = mybir.AluOpType
    Act = mybir.ActivationFunctionType

    cN, cD, cDEC, cNC, cNP, cBLK, cNB, cP = (
        cfg["N"], cfg["D"], cfg["DEC"], cfg["NC"], cfg["NP"], cfg["BLK"],
        cfg["NB"], cfg["P"])
    AG = [list(range(cNC))]

    nc = bacc.Bacc("TRN2", target_bir_lowering=False, debug=False,
                   num_devices=cNC)

    def din(name, shape, dt):
        return nc.dram_tensor(name, shape, dt, kind="ExternalInput").ap()

    x_own = din("x_own", [cNP, cD], bf16)
    m1_own = din("m1_own", [cNP, cD], bf16)
    m2_own = din("m2_own", [cNP, cD], bf16)
    src_e = din("src_e", [cP, cNB * T], i32)
    dst_e = din("dst_e", [cP, cNB * T], f32)
    scl_own = din("scl_own", [cBLK, cNB], f32)
    shuf = din("shuf", [cP, NTI], i32)
    w1t = din("w1t", [cD, cD], bf16)
    w2t = din("w2t", [cD, cD], bf16)
    wdt = din("wdt", [cD, cDEC], bf16)
    b1bc = din("b1bc", [cP, cD], f32)
    b2bc = din("b2bc", [cP, cD], f32)
    bdbc = din("bdbc", [cP, cDEC], f32)
    iota_in = din("iota", [cP, cBLK], f32)

    h2_out = nc.dram_tensor("h2_out", [cNP, cD], bf16,
                            kind="ExternalOutput").ap()
    dec_out = nc.dram_tensor("dec_out", [1, 1], f32,
                             kind="ExternalOutput").ap()
    if debug:
        dbg_h0 = nc.dram_tensor("dbg_h0", [cNP, cD], f32,
                                kind="ExternalOutput").ap()
        dbg_msg = nc.dram_tensor("dbg_msg", [cP, T * cD], f32,
                                 kind="ExternalOutput").ap()
        dbg_mp = nc.dram_tensor("dbg_mp", [cP, T * cBLK], f32,
                                kind="ExternalOutput").ap()
        dbg_nT = nc.dram_tensor("dbg_nT", [cD, cBLK], f32,
                                kind="ExternalOutput").ap()
        dbg_h1 = nc.dram_tensor("dbg_h1", [cNP, cD], f32,
                                kind="ExternalOutput").ap()
        dbg_z = nc.dram_tensor("dbg_z", [cNP, cDEC], f32,
                               kind="ExternalOutput").ap()
        dbg_ls = nc.dram_tensor("dbg_ls", [cNP, cDEC], f32,
                                kind="ExternalOutput").ap()

    with tile.TileContext(nc) as tc:
        with (
            tc.tile_pool(name="cst", bufs=1) as cst,
            tc.tile_pool(name="big", bufs=1) as big,
            tc.tile_pool(name="sb", bufs=2) as sb,
            tc.tile_pool(name="psn", bufs=2, space="PSUM") as psn,
            tc.tile_pool(name="psf", bufs=2, space="PSUM") as psf,
            tc.tile_pool(name="pst", bufs=1, space="PSUM") as pst,
            tc.tile_pool(name="psz", bufs=1, space="PSUM") as psz,
            tc.tile_pool(name="dram", bufs=1, space="DRAM") as dram,
        ):
            # ---- persistent constants ----
            srcs = cst.tile([cP, cNB * T], i32)
            nc.sync.dma_start(out=srcs[:], in_=src_e[:, :])
            dsts = cst.tile([cP, cNB * T], f32)
            nc.sync.dma_start(out=dsts[:], in_=dst_e[:, :])
            iot = cst.tile([cP, cBLK], f32)
            nc.sync.dma_start(out=iot[:], in_=iota_in[:, :])
            sclc = cst.tile([cBLK, cNB], f32)
            nc.sync.dma_start(out=sclc[:], in_=scl_own[:, :])
            w1 = cst.tile([cD, cD], bf16)
            nc.sync.dma_start(out=w1[:], in_=w1t[:, :])
            w2 = cst.tile([cD, cD], bf16)
            nc.sync.dma_start(out=w2[:], in_=w2t[:, :])
            wd = cst.tile([cD, cDEC], bf16)
            nc.sync.dma_start(out=wd[:], in_=wdt[:, :])
            bb1 = cst.tile([cP, cD], f32)
            nc.sync.dma_start(out=bb1[:], in_=b1bc[:, :])
            bb2 = cst.tile([cP, cD], f32)
            nc.sync.dma_start(out=bb2[:], in_=b2bc[:, :])
            bbd = cst.tile([cP, cDEC], f32)
            nc.sync.dma_start(out=bbd[:], in_=bdbc[:, :])
            shufs = cst.tile([cP, NTI], i32)
            nc.sync.dma_start(out=shufs[:], in_=shuf[:, :])
            ident = cst.tile([cP, cP], f32)
            make_identity(nc, ident[:])
            m2res = cst.tile([cBLK, cNB * cD], bf16)
            nc.sync.dma_start(
                out=m2res[:].rearrange("n (b d) -> n b d", b=cNB),
                in_=m2_own.rearrange("(b n) d -> n b d", n=cBLK))

            # persistent per-layer result accumulators
            h1res = cst.tile([cBLK, cNB * cD], bf16)
            h2res = cst.tile([cBLK, cNB * cD], f32)
            zres = cst.tile([cBLK, cNB * cDEC], bf16)
            lsres = cst.tile([cBLK, cNB * cDEC], f32)

            # ---- DRAM intermediates ----
            h0b = dram.tile([cNP, cD], bf16)
            h0f = dram.tile([cN, cD], bf16)
            h1b = dram.tile([cNP, cD], bf16)
            h1f = dram.tile([cN, cD], bf16)
            zb = dram.tile([cNP, cDEC], bf16)
            zf = dram.tile([cN, cDEC], bf16)
            lsd = dram.tile([NTI * cP, cDEC], f32)

            # ---- phase A: h0 = x * mask1 (bf16 table) ----
            xres = big.tile([cBLK, cNB * cD], bf16, tag="big20")
            nc.sync.dma_start(
                out=xres[:].rearrange("n (b d) -> n b d", b=cNB),
                in_=x_own.rearrange("(b n) d -> n b d", n=cBLK))
            m1res = big.tile([cBLK, cNB * cD], bf16, tag="bigA")
            nc.sync.dma_start(
                out=m1res[:].rearrange("n (b d) -> n b d", b=cNB),
                in_=m1_own.rearrange("(b n) d -> n b d", n=cBLK))
            h0res = big.tile([cBLK, cNB * cD], bf16, tag="bigB")
            nc.vector.tensor_tensor(out=h0res[:], in0=xres[:], in1=m1res[:],
                                    op=Alu.mult)
            nc.sync.dma_start(
                out=h0b[:].rearrange("(b n) d -> n b d", n=cBLK),
                in_=h0res[:].rearrange("n (b d) -> n b d", b=cNB))
            if debug:
                nc.gpsimd.dma_start(
                    out=dbg_h0.rearrange("(b n) d -> n b d", n=cBLK),
                    in_=h0res[:].rearrange("n (b d) -> n b d", b=cNB))
            nc.gpsimd.collective_compute(
                "AllGather", Alu.bypass, replica_groups=AG,
                ins=[h0b[:].opt()], outs=[h0f[:].opt()])

            # ---- shared aggregation + FC ----
            def emit_layer(table, wt, layer_idx):
                for b in range(cNB):
                    # per-tile [128,1] gathers of this block's edge sources
                    # (the only indirect-DMA form the HW handles correctly)
                    msg = sb.tile([cP, T * cD], bf16, tag="msg")
                    msgv0 = msg[:].rearrange("p (t d) -> p t d", t=T)
                    for t in range(T):
                        nc.gpsimd.indirect_dma_start(
                            out=msgv0[:, t, :],
                            out_offset=None,
                            in_=table[:],
                            in_offset=bass.IndirectOffsetOnAxis(
                                ap=srcs[:, b * T + t:b * T + t + 1], axis=0),
                        )
                    # selection matrix M[e, t, s] = (dst_local[e,t] == s)
                    mp = sb.tile([cP, T * cBLK], bf16, tag="mp")
                    dstb = dsts[:, b * T:(b + 1) * T].to_broadcast(
                        [cP, T, cBLK])
                    iob = iot[:].rearrange("p (o k) -> p o k", o=1)\
                        .to_broadcast([cP, T, cBLK])
                    nc.vector.tensor_tensor(
                        out=mp[:].rearrange("p (t k) -> p t k", t=T),
                        in0=dstb, in1=iob, op=Alu.is_equal)
                    # neigh^T[f, s] accumulated over edge tiles
                    pn = psn.tile([cD, cBLK], mybir.dt.float32, tag="pn")
                    msgv = msg[:].rearrange("p (t d) -> p t d", t=T)
                    mpv = mp[:].rearrange("p (t k) -> p t k", t=T)
                    for t in range(T):
                        nc.tensor.matmul(
                            pn[:], lhsT=msgv[:, t, :], rhs=mpv[:, t, :],
                            start=(t == 0), stop=(t == T - 1))
                    nT = sb.tile([cD, cBLK], bf16, tag="nT")
                    nc.vector.tensor_copy(out=nT[:], in_=pn[:])
                    if debug and layer_idx == 1 and b == 0:
                        nc.gpsimd.dma_start(out=dbg_msg[:, :], in_=msg[:])
                        nc.gpsimd.dma_start(out=dbg_mp[:, :], in_=mp[:])
                        nc.gpsimd.dma_start(out=dbg_nT[:, :], in_=nT[:])
                    # FC: out[n, o] = neigh^T.T @ W^T  (scale folded after)
                    pf = psf.tile([cBLK, cD], mybir.dt.float32, tag="pf")
                    nc.tensor.matmul(pf[:], lhsT=nT[:], rhs=wt[:],
                                     start=True, stop=True)
                    t1 = sb.tile([cBLK, cD], f32, tag="t1")
                    nc.vector.tensor_scalar(
                        out=t1[:], in0=pf[:], scalar1=sclc[:, b:b + 1],
                        scalar2=None, op0=Alu.mult)
                    if layer_idx == 1:
                        t2 = sb.tile([cBLK, cD], f32, tag="t2")
                        nc.vector.tensor_tensor(out=t2[:], in0=t1[:],
                                                in1=bb1[:cBLK, :], op=Alu.add)
                        # relu(x)*m == relu(x*m) since m >= 0
                        t3 = sb.tile([cBLK, cD], f32, tag="t3")
                        m2v = m2res[:].rearrange("n (b d) -> n b d", b=cNB)
                        nc.vector.tensor_tensor(out=t3[:], in0=t2[:],
                                                in1=m2v[:, b, :], op=Alu.mult)
                        h1v = h1res[:].rearrange("n (b d) -> n b d", b=cNB)
                        nc.vector.tensor_scalar(
                            out=h1v[:, b, :], in0=t3[:], scalar1=0.0,
                            scalar2=None, op0=Alu.max)
                    else:
                        h2v = h2res[:].rearrange("n (b d) -> n b d", b=cNB)
                        nc.vector.tensor_tensor(out=h2v[:, b, :], in0=t1[:],
                                                in1=bb2[:cBLK, :], op=Alu.add)
                        # transpose h2 block for the decoder FC
                        pt = pst.tile([cD, cBLK], mybir.dt.float32, tag="pt")
                        nc.tensor.transpose(pt[:], h2v[:, b, :],
                                            ident[:cBLK, :cBLK])
                        hT = sb.tile([cD, cBLK], bf16, tag="hT")
                        nc.vector.tensor_copy(out=hT[:], in_=pt[:])
                        pz = psz.tile([cBLK, cDEC], mybir.dt.float32,
                                      tag="pz")
                        nc.tensor.matmul(pz[:], lhsT=hT[:], rhs=wd[:],
                                         start=True, stop=True)
                        zsb = sb.tile([cBLK, cDEC], f32, tag="zsb")
                        nc.vector.tensor_tensor(out=zsb[:], in0=pz[:],
                                                in1=bbd[:cBLK, :], op=Alu.add)
                        zv = zres[:].rearrange("n (b d) -> n b d", b=cNB)
                        nc.vector.tensor_copy(out=zv[:, b, :], in_=zsb[:])
                        # log-softmax over DEC
                        negm = sb.tile([cBLK, 1], f32, tag="negm")
                        nc.vector.tensor_reduce(
                            out=negm[:], in_=zsb[:],
                            axis=mybir.AxisListType.X, op=Alu.max,
                            negate=True)
                        esb = sb.tile([cBLK, cDEC], f32, tag="esb")
                        nc.scalar.activation(esb[:], zsb[:], Act.Exp,
                                             bias=negm[:, 0:1], scale=1.0)
                        ssum = sb.tile([cBLK, 1], f32, tag="ssum")
                        nc.vector.tensor_reduce(
                            out=ssum[:], in_=esb[:],
                            axis=mybir.AxisListType.X, op=Alu.add)
                        lns = sb.tile([cBLK, 1], f32, tag="lns")
                        nc.scalar.activation(lns[:], ssum[:], Act.Ln)
                        mpl = sb.tile([cBLK, 1], f32, tag="mpl")
                        nc.vector.tensor_tensor(out=mpl[:], in0=lns[:],
                                                in1=negm[:], op=Alu.subtract)
                        lsv = lsres[:].rearrange("n (b d) -> n b d", b=cNB)
                        nc.vector.tensor_scalar(
                            out=lsv[:, b, :], in0=zsb[:], scalar1=mpl[:, 0:1],
                            scalar2=None, op0=Alu.subtract)

            # ---- layer 1 ----
            emit_layer(h0f, w1, 1)
            nc.sync.dma_start(
                out=h1b[:].rearrange("(b n) d -> n b d", n=cBLK),
                in_=h1res[:].rearrange("n (b d) -> n b d", b=cNB))
            if debug:
                nc.gpsimd.dma_start(
                    out=dbg_h1.rearrange("(b n) d -> n b d", n=cBLK),
                    in_=h1res[:].rearrange("n (b d) -> n b d", b=cNB))
            nc.gpsimd.collective_compute(
                "AllGather", Alu.bypass, replica_groups=AG,
                ins=[h1b[:].opt()], outs=[h1f[:].opt()])

            # ---- layer 2 + decoder per-block ----
            emit_layer(h1f, w2, 2)
            h2bf = big.tile([cBLK, cNB * cD], bf16, tag="bigC")
            nc.vector.tensor_copy(out=h2bf[:], in_=h2res[:])
            nc.sync.dma_start(
                out=h2_out.rearrange("(b n) d -> n b d", n=cBLK),
                in_=h2bf[:].rearrange("n (b d) -> n b d", b=cNB))
            nc.sync.dma_start(
                out=zb[:].rearrange("(b n) d -> n b d", n=cBLK),
                in_=zres[:].rearrange("n (b d) -> n b d", b=cNB))
            nc.gpsimd.collective_compute(
                "AllGather", Alu.bypass, replica_groups=AG,
                ins=[zb[:].opt()], outs=[zf[:].opt()])
            if debug:
                nc.gpsimd.dma_start(
                    out=dbg_z.rearrange("(b n) d -> n b d", n=cBLK),
                    in_=zres[:].rearrange("n (b d) -> n b d", b=cNB))
                nc.sync.dma_start(
                    out=dbg_ls.rearrange("(b n) d -> n b d", n=cBLK),
                    in_=lsres[:].rearrange("n (b d) -> n b d", b=cNB))
            nc.sync.dma_start(
                out=lsd[0:cNP, :].rearrange("(b n) d -> n b d", n=cBLK),
                in_=lsres[:].rearrange("n (b d) -> n b d", b=cNB))
            pad_rows = NTI * cP - cNP
            if pad_rows:
                zpad = sb.tile([pad_rows, cDEC], f32, tag="zpad")
                nc.vector.memset(zpad[:], 0.0)
                nc.sync.dma_start(out=lsd[cNP:, :], in_=zpad[:])

            # ---- decoder tail ----
            lst = big.tile([cP, NTI * cDEC], f32, tag="bigA")
            nc.sync.dma_start(
                out=lst[:].rearrange("p (i d) -> p i d", i=NTI),
                in_=lsd[:].rearrange("(i p) d -> p i d", p=cP))
            zg = big.tile([cP, NTI * cDEC], bf16, tag="bigB")
            zgv = zg[:].rearrange("p (i d) -> p i d", i=NTI)
            for i in range(NTI):
                nc.gpsimd.indirect_dma_start(
                    out=zgv[:, i, :],
                    out_offset=None,
                    in_=zf[:],
                    in_offset=bass.IndirectOffsetOnAxis(
                        ap=shufs[:, i:i + 1], axis=0),
                )
            pr = big.tile([cP, NTI * cDEC], f32, tag="big20")
            nc.vector.tensor_tensor(out=pr[:], in0=zg[:], in1=lst[:],
                                    op=Alu.mult)
            ac = sb.tile([cP, 1], f32, tag="ac")
            nc.vector.tensor_reduce(
                out=ac[:], in_=pr[:].rearrange("p (i d) -> p i d", i=NTI),
                axis=mybir.AxisListType.XY, op=Alu.add)
            ones = cst.tile([cP, 1], f32)
            nc.vector.memset(ones[:], 1.0)
            pd = psz.tile([1, 1], mybir.dt.float32, tag="pd")
            nc.tensor.matmul(pd[:], lhsT=ac[:], rhs=ones[:], start=True,
                             stop=True)
            dsb = sb.tile([1, 1], f32, tag="dsb")
            nc.vector.tensor_copy(out=dsb[:], in_=pd[:])
            nc.sync.dma_start(out=dec_out[:, :], in_=dsb[:])

    nc.compile()
    return nc


# ---- host-side sharding / preprocessing --------------------------------------

def host_prep(cfg, x, src, dst, shuffled_index, W1, b1, W2, b2, Wd, bd,
              mask1, mask2):
    """Partition the graph and pack per-core DMA-friendly arrays.
    Returns (in_maps, T, NTI)."""
    cN, cD, cDEC, cNC, cNP, cBLK, cNB, cP = (
        cfg["N"], cfg["D"], cfg["DEC"], cfg["NC"], cfg["NP"], cfg["BLK"],
        cfg["NB"], cfg["P"])
    bf = ml_dtypes.bfloat16

    src = np.asarray(src).astype(np.int64)
    dst = np.asarray(dst).astype(np.int64)
    shuffled_index = np.asarray(shuffled_index).astype(np.int64)
    x = np.ascontiguousarray(np.asarray(x, dtype=np.float32))

    deg = np.bincount(dst, minlength=cN)
    scale = (1.0 / (deg + 1.0)).astype(np.float32)

    # sort edges by dst; blocks never straddle cores since NP % BLK == 0
    order = np.argsort(dst, kind="stable")
    s_src = src[order]
    s_dst = dst[order]
    gblk = s_dst // cBLK                      # global block id, 0..NC*NB-1
    nblk = cNC * cNB
    counts = np.bincount(gblk, minlength=nblk)
    ptr = np.zeros(nblk + 1, np.int64)
    np.cumsum(counts, out=ptr[1:])
    tot = counts + cBLK                       # + self edges
    T = int(np.ceil(tot.max() / cP))
    slots = T * cP

    src_all = np.zeros(nblk * slots, np.int32)          # pad -> row 0
    dst_all = np.full(nblk * slots, -1.0, np.float32)   # pad -> no slot
    # real edges
    rank = np.arange(len(s_src)) - ptr[gblk]
    pos = gblk * slots + rank
    src_all[pos] = s_src
    dst_all[pos] = (s_dst % cBLK).astype(np.float32)
    # self edges
    g = np.arange(nblk)
    vbase = (g // cNB) * cNP + (g % cNB) * cBLK
    selfpos = (g[:, None] * slots + counts[:, None]
               + np.arange(cBLK)[None, :]).ravel()
    src_all[selfpos] = (vbase[:, None] + np.arange(cBLK)[None, :]).ravel()
    dst_all[selfpos] = np.tile(np.arange(cBLK, dtype=np.float32), nblk)

    # [core, (b t p)] -> [core][P, NB*T] with col = b*T+t, lane p
    src_all = src_all.reshape(cNC, cNB * T, cP)
    dst_all = dst_all.reshape(cNC, cNB * T, cP)

    NTI = -(-cNP // cP)
    shuf_pad = np.zeros(cNC * NTI * cP, np.int32)
    shuf_pad[: cNC * cNP] = shuffled_index.reshape(cNC, cNP).ravel()
    # careful: padding must be per core
    shuf_pc = np.zeros((cNC, NTI * cP), np.int32)
    shuf_pc[:, :cNP] = shuffled_index.reshape(cNC, cNP)
    shuf_pc = shuf_pc.reshape(cNC, NTI, cP)

    scale_pc = scale.reshape(cNC, cNB, cBLK)

    w1t = np.ascontiguousarray(np.asarray(W1, np.float32).T).astype(bf)
    w2t = np.ascontiguousarray(np.asarray(W2, np.float32).T).astype(bf)
    wdt = np.ascontiguousarray(np.asarray(Wd, np.float32).T).astype(bf)
    b1bc = np.ascontiguousarray(
        np.broadcast_to(np.asarray(b1, np.float32), (cP, cD)))
    b2bc = np.ascontiguousarray(
        np.broadcast_to(np.asarray(b2, np.float32), (cP, cD)))
    bdbc = np.ascontiguousarray(
        np.broadcast_to(np.asarray(bd, np.float32), (cP, cDEC)))
    iota = np.ascontiguousarray(
        np.broadcast_to(np.arange(cBLK, dtype=np.float32), (cP, cBLK)))

    m1 = np.asarray(mask1, np.float32).astype(bf).reshape(cNC, cNP, cD)
    xbf = x.astype(bf)
    m2 = np.asarray(mask2, np.float32).astype(bf).reshape(cNC, cNP, cD)
    xs = xbf.reshape(cNC, cNP, cD)

    in_maps = []
    for c in range(cNC):
        in_maps.append({
            "x_own": np.ascontiguousarray(xs[c]),
            "m1_own": np.ascontiguousarray(m1[c]),
            "m2_own": np.ascontiguousarray(m2[c]),
            "src_e": np.ascontiguousarray(src_all[c].T),
            "dst_e": np.ascontiguousarray(dst_all[c].T),
            "scl_own": np.ascontiguousarray(scale_pc[c].T),
            "shuf": np.ascontiguousarray(
                shuf_pc[c].T.reshape(cP, NTI)),
            "w1t": w1t, "w2t": w2t, "wdt": wdt,
            "b1bc": b1bc, "b2bc": b2bc, "bdbc": bdbc,
            "iota": iota,
        })
    return in_maps, T, NTI


# ---- execution ---------------------------------------------------------------

class PjrtRunner:
    """Mirror of bass2jax.run_bass_via_pjrt that keeps the jitted callable
    so repeat executions skip retracing."""

    def __init__(self, nc, n_cores):
        import jax
        import numpy as np
        from jax.sharding import Mesh, PartitionSpec
        from jax.experimental.shard_map import shard_map
        import concourse.mybir as mybir
        from concourse import bass2jax

        bass2jax.install_neuronx_cc_hook()
        self.jax = jax
        self.nc = nc
        self.n_cores = n_cores
        partition_name = (nc.partition_id_tensor.name
                          if nc.partition_id_tensor else None)
        in_names, out_names, out_avals, zero_outs = [], [], [], []
        for alloc in nc.m.functions[0].allocations:
            if not isinstance(alloc, mybir.MemoryLocationSet):
                continue
            name = alloc.memorylocations[0].name
            if alloc.kind == "ExternalInput":
                if name != partition_name:
                    in_names.append(name)
            elif alloc.kind == "ExternalOutput":
                shape = tuple(alloc.tensor_shape)
                dtype = mybir.dt.np(alloc.dtype)
                out_names.append(name)
                out_avals.append(jax.core.ShapedArray(shape, dtype))
                zero_outs.append(np.zeros(shape, dtype))
        self.in_names = in_names
        self.out_names = out_names
        self.out_avals = out_avals
        self.zero_outs = zero_outs
        n_params = len(in_names)
        n_outs = len(out_avals)
        all_names = list(in_names) + list(out_names)
        if partition_name is not None:
            all_names.append(partition_name)

        def _body(*args):
            operands = list(args)
            if partition_name is not None:
                operands.append(bass2jax.partition_id_tensor())
            outs = bass2jax._bass_exec_p.bind(
                *operands, out_avals=tuple(out_avals),
                in_names=tuple(all_names), out_names=tuple(out_names),
                lowering_input_output_aliases=(),
                sim_require_finite=True, sim_require_nnan=True, nc=nc)
            return tuple(outs)

        devices = jax.devices()[:n_cores]
        mesh = Mesh(np.asarray(devices), ("core",))
        in_specs = (PartitionSpec("core"),) * (n_params + n_outs)
        out_specs = (PartitionSpec("core"),) * n_outs
        self.sharded = jax.jit(
            shard_map(_body, mesh=mesh, in_specs=in_specs,
                      out_specs=out_specs, check_rep=False),
            keep_unused=True)
        self._dev_args = None

    def put(self, in_maps):
        np_ = np
        concat_in = [
            np_.concatenate([np_.asarray(in_maps[c][nm])
                             for c in range(self.n_cores)], axis=0)
            for nm in self.in_names]
        concat_zeros = [
            np_.zeros((self.n_cores * z.shape[0], *z.shape[1:]), z.dtype)
            for z in self.zero_outs]
        self._dev_args = [self.jax.device_put(a)
                          for a in concat_in + concat_zeros]

    def run(self):
        outs = self.sharded(*self._dev_args)
        self.jax.block_until_ready(outs)
        return [
            {nm: np.asarray(outs[i]).reshape(
                self.n_cores, *self.out_avals[i].shape)[c]
             for i, nm in enumerate(self.out_names)}
            for c in range(self.n_cores)]


def _compute_masks(shape1, shape2):
    import jax
    cpu = jax.devices("cpu")[0]
    with jax.default_device(cpu):
        dk = jax.random.split(jax.random.key(42), 2)
        m1 = np.asarray(
            jax.random.bernoulli(dk[0], 0.5, shape1)).astype(np.float32) * 2.0
        m2 = np.asarray(
            jax.random.bernoulli(dk[1], 0.5, shape2)).astype(np.float32) * 2.0
    return m1, m2


def kernel(x, src, dst, shuffled_index, W1, b1, W2, b2, Wd, bd):
    cfg = _FULL_CFG
    mask1, mask2 = _compute_masks((cfg["N"], cfg["D"]), (cfg["N"], cfg["D"]))
    in_maps, T, NTI = host_prep(cfg, x, src, dst, shuffled_index,
                                W1, b1, W2, b2, Wd, bd, mask1, mask2)
    key = (T, NTI)
    if key not in _program_cache:
        nc = build_program(cfg, T, NTI)
        _program_cache[key] = PjrtRunner(nc, cfg["NC"])
    runner = _program_cache[key]
    runner.put(in_maps)
    res = runner.run()
    h = np.concatenate([np.asarray(res[c]["h2_out"]).astype(np.float32)
                        for c in range(cfg["NC"])], axis=0)
    dec = -sum(float(res[c]["dec_out"][0, 0]) for c in range(cfg["NC"])) \
        / cfg["N"]
    return h.astype(np.float32), np.float32(dec)


# revision 11
# speedup vs baseline: 1.3662x; 1.0027x over previous
"""Trainium2 Bass kernel for nn_Encoder_14018773254741 (2-layer SAGE-GCN
encoder + soft-target CE decoder) on 8 NeuronCores.

Strategy (1D node partition):
  - 40000 nodes split 5000/core; each core owns the edges whose dst lands in
    its range (plus one explicit self-edge per node, which implements the
    "+ h_v" term of the gcn aggregator).
  - Edges are sorted by dst and packed into fixed-size tiles of 128 on the
    host; per 125-node block the aggregation runs as: batched indirect-DMA
    gather of h[src] rows (bf16 table) -> one-hot selection matrix built
    on-chip (iota == dst_local) -> PE matmuls accumulating neigh^T in PSUM.
  - The 128-dim linears replicate weights; deg-scaling is folded in after
    the FC ((S*A) @ W == S*(A @ W)); dropout masks are precomputed constants
    (fixed jax key) applied on-chip.
  - Halo exchange: the full (quantized bf16) node-feature table is
    AllGather-ed between layers; decoder logits z are AllGather-ed once for
    the shuffled-row gather.

Outputs: h [40000,128] f32 (node-major shards concatenated on host) and the
scalar dec (per-core partial sums, final mean on host).
"""

import numpy as np
import ml_dtypes

# ---- hardcoded problem config ------------------------------------------------
N = 40000          # nodes
D = 128            # feature dim (in = hid = out)
DEC = 64           # decoder dim
NC = 8             # cores
NP = N // NC       # nodes per core
BLK = 125          # dst nodes per aggregation block
NB = NP // BLK     # blocks per core
P = 128            # partitions / edge-tile size

_FULL_CFG = dict(N=N, D=D, DEC=DEC, NC=NC, NP=NP, BLK=BLK, NB=NB, P=P)

_program_cache = {}


# ---- device program ----------------------------------------------------------

def build_program(cfg, T, NTI, debug=False):
    """Build the 8-core SPMD Bass program. T = edge tiles per block,
    NTI = decoder gather tiles (= ceil(NP/128))."""
    import concourse.bass as bass
    import concourse.bacc as bacc
    import concourse.mybir as mybir
    import concourse.tile as tile
    from concourse.masks import make_identity

    f32 = mybir.dt.float32
    bf16 = mybir.dt.bfloat16
    i32 = mybir.dt.int32
    Alu = mybir.AluOpType
    Act = mybir.ActivationFunctionType

    cN, cD, cDEC, cNC, cNP, cBLK, cNB, cP = (
        cfg["N"], cfg["D"], cfg["DEC"], cfg["NC"], cfg["NP"], cfg["BLK"],
        cfg["NB"], cfg["P"])
    AG = [list(range(cNC))]

    nc = bacc.Bacc("TRN2", target_bir_lowering=False, debug=False,
                   num_devices=cNC)

    def din(name, shape, dt):
        return nc.dram_tensor(name, shape, dt, kind="ExternalInput").ap()

    x_own = din("x_own", [cNP, cD], bf16)
    m1_own = din("m1_own", [cNP, cD], mybir.dt.int8)
    m2_own = din("m2_own", [cNP, cD], mybir.dt.int8)
    src_e = din("src_e", [cP, cNB * T], i32)
    dst_e = din("dst_e", [cP, cNB * T], mybir.dt.int8)
    scl_own = din("scl_own", [cBLK, cNB], f32)
    shuf = din("shuf", [cP, NTI], i32)
    w1t = din("w1t", [cD, cD], bf16)
    w2t = din("w2t", [cD, cD], bf16)
    wdt = din("wdt", [cD, cDEC], bf16)
    b1bc = din("b1bc", [cP, cD], f32)
    b2bc = din("b2bc", [cP, cD], f32)
    bdbc = din("bdbc", [cP, cDEC], f32)
    iota_in = din("iota", [cP, cBLK], f32)

    h2_out = nc.dram_tensor("h2_out", [cNP, cD], bf16,
                            kind="ExternalOutput").ap()
    dec_out = nc.dram_tensor("dec_out", [1, 1], f32,
                             kind="ExternalOutput").ap()
    if debug:
        dbg_h0 = nc.dram_tensor("dbg_h0", [cNP, cD], f32,
                                kind="ExternalOutput").ap()
        dbg_msg = nc.dram_tensor("dbg_msg", [cP, T * cD], f32,
                                 kind="ExternalOutput").ap()
        dbg_mp = nc.dram_tensor("dbg_mp", [cP, T * cBLK], f32,
                                kind="ExternalOutput").ap()
        dbg_nT = nc.dram_tensor("dbg_nT", [cD, cBLK], f32,
                                kind="ExternalOutput").ap()
        dbg_h1 = nc.dram_tensor("dbg_h1", [cNP, cD], f32,
                                kind="ExternalOutput").ap()
        dbg_z = nc.dram_tensor("dbg_z", [cNP, cDEC], f32,
                               kind="ExternalOutput").ap()
        dbg_ls = nc.dram_tensor("dbg_ls", [cNP, cDEC], f32,
                                kind="ExternalOutput").ap()

    with tile.TileContext(nc) as tc:
        with (
            tc.tile_pool(name="cst", bufs=1) as cst,
            tc.tile_pool(name="big", bufs=1) as big,
            tc.tile_pool(name="sb", bufs=2) as sb,
            tc.tile_pool(name="psn", bufs=2, space="PSUM") as psn,
            tc.tile_pool(name="psf", bufs=2, space="PSUM") as psf,
            tc.tile_pool(name="pst", bufs=1, space="PSUM") as pst,
            tc.tile_pool(name="psz", bufs=1, space="PSUM") as psz,
            tc.tile_pool(name="dram", bufs=1, space="DRAM") as dram,
        ):
            # ---- persistent constants ----
            srcs = cst.tile([cP, cNB * T], i32)
            nc.sync.dma_start(out=srcs[:], in_=src_e[:, :])
            dsts = cst.tile([cP, cNB * T], mybir.dt.int8)
            nc.sync.dma_start(out=dsts[:], in_=dst_e[:, :])
            iot = cst.tile([cP, cBLK], f32)
            nc.sync.dma_start(out=iot[:], in_=iota_in[:, :])
            sclc = cst.tile([cBLK, cNB], f32)
            nc.sync.dma_start(out=sclc[:], in_=scl_own[:, :])
            w1 = cst.tile([cD, cD], bf16)
            nc.sync.dma_start(out=w1[:], in_=w1t[:, :])
            w2 = cst.tile([cD, cD], bf16)
            nc.sync.dma_start(out=w2[:], in_=w2t[:, :])
            wd = cst.tile([cD, cDEC], bf16)
            nc.sync.dma_start(out=wd[:], in_=wdt[:, :])
            bb1 = cst.tile([cP, cD], f32)
            nc.sync.dma_start(out=bb1[:], in_=b1bc[:, :])
            bb2 = cst.tile([cP, cD], f32)
            nc.sync.dma_start(out=bb2[:], in_=b2bc[:, :])
            bbd = cst.tile([cP, cDEC], f32)
            nc.sync.dma_start(out=bbd[:], in_=bdbc[:, :])
            shufs = cst.tile([cP, NTI], i32)
            nc.sync.dma_start(out=shufs[:], in_=shuf[:, :])
            ident = cst.tile([cP, cP], f32)
            make_identity(nc, ident[:])
            m2res = cst.tile([cBLK, cNB * cD], mybir.dt.int8)
            nc.sync.dma_start(
                out=m2res[:].rearrange("n (b d) -> n b d", b=cNB),
                in_=m2_own.rearrange("(b n) d -> n b d", n=cBLK))

            # persistent per-layer result accumulators
            h1res = cst.tile([cBLK, cNB * cD], bf16)
            h2res = cst.tile([cBLK, cNB * cD], f32)
            zres = cst.tile([cBLK, cNB * cDEC], bf16)
            lsres = cst.tile([cBLK, cNB * cDEC], f32)

            # ---- DRAM intermediates ----
            h0b = dram.tile([cNP, cD], bf16)
            h0f = dram.tile([cN, cD], bf16)
            h1b = dram.tile([cNP, cD], bf16)
            h1f = dram.tile([cN, cD], bf16)
            zb = dram.tile([cNP, cDEC], bf16)
            zf = dram.tile([cN, cDEC], bf16)
            lsd = dram.tile([NTI * cP, cDEC], f32)

            # ---- phase A: h0 = x * mask1 (bf16 table) ----
            xres = big.tile([cBLK, cNB * cD], bf16, tag="big20")
            nc.sync.dma_start(
                out=xres[:].rearrange("n (b d) -> n b d", b=cNB),
                in_=x_own.rearrange("(b n) d -> n b d", n=cBLK))
            m1res = big.tile([cBLK, cNB * cD], mybir.dt.int8, tag="bigA")
            nc.sync.dma_start(
                out=m1res[:].rearrange("n (b d) -> n b d", b=cNB),
                in_=m1_own.rearrange("(b n) d -> n b d", n=cBLK))
            h0res = big.tile([cBLK, cNB * cD], bf16, tag="bigB")
            nc.vector.tensor_tensor(out=h0res[:], in0=xres[:], in1=m1res[:],
                                    op=Alu.mult)
            nc.sync.dma_start(
                out=h0b[:].rearrange("(b n) d -> n b d", n=cBLK),
                in_=h0res[:].rearrange("n (b d) -> n b d", b=cNB))
            if debug:
                nc.gpsimd.dma_start(
                    out=dbg_h0.rearrange("(b n) d -> n b d", n=cBLK),
                    in_=h0res[:].rearrange("n (b d) -> n b d", b=cNB))
            nc.gpsimd.collective_compute(
                "AllGather", Alu.bypass, replica_groups=AG,
                ins=[h0b[:].opt()], outs=[h0f[:].opt()])

            # ---- shared aggregation + FC ----
            def emit_layer(table, wt, layer_idx):
                for b in range(cNB):
                    # per-tile [128,1] gathers of this block's edge sources
                    # (the only indirect-DMA form the HW handles correctly)
                    msg = sb.tile([cP, T * cD], bf16, tag="msg")
                    msgv0 = msg[:].rearrange("p (t d) -> p t d", t=T)
                    for t in range(T):
                        nc.gpsimd.indirect_dma_start(
                            out=msgv0[:, t, :],
                            out_offset=None,
                            in_=table[:],
                            in_offset=bass.IndirectOffsetOnAxis(
                                ap=srcs[:, b * T + t:b * T + t + 1], axis=0),
                        )
                    # selection matrix M[e, t, s] = (dst_local[e,t] == s)
                    mp = sb.tile([cP, T * cBLK], bf16, tag="mp")
                    dstb = dsts[:, b * T:(b + 1) * T].to_broadcast(
                        [cP, T, cBLK])
                    iob = iot[:].rearrange("p (o k) -> p o k", o=1)\
                        .to_broadcast([cP, T, cBLK])
                    nc.vector.tensor_tensor(
                        out=mp[:].rearrange("p (t k) -> p t k", t=T),
                        in0=dstb, in1=iob, op=Alu.is_equal)
                    # neigh^T[f, s] accumulated over edge tiles
                    pn = psn.tile([cD, cBLK], mybir.dt.float32, tag="pn")
                    msgv = msg[:].rearrange("p (t d) -> p t d", t=T)
                    mpv = mp[:].rearrange("p (t k) -> p t k", t=T)
                    for t in range(T):
                        nc.tensor.matmul(
                            pn[:], lhsT=msgv[:, t, :], rhs=mpv[:, t, :],
                            start=(t == 0), stop=(t == T - 1))
                    nT = sb.tile([cD, cBLK], bf16, tag="nT")
                    nc.vector.tensor_copy(out=nT[:], in_=pn[:])
                    if debug and layer_idx == 1 and b == 0:
                        nc.gpsimd.dma_start(out=dbg_msg[:, :], in_=msg[:])
                        nc.gpsimd.dma_start(out=dbg_mp[:, :], in_=mp[:])
                        nc.gpsimd.dma_start(out=dbg_nT[:, :], in_=nT[:])
                    # FC: out[n, o] = neigh^T.T @ W^T  (scale folded after)
                    pf = psf.tile([cBLK, cD], mybir.dt.float32, tag="pf")
                    nc.tensor.matmul(pf[:], lhsT=nT[:], rhs=wt[:],
                                     start=True, stop=True)
                    t1 = sb.tile([cBLK, cD], f32, tag="t1")
                    nc.vector.tensor_scalar(
                        out=t1[:], in0=pf[:], scalar1=sclc[:, b:b + 1],
                        scalar2=None, op0=Alu.mult)
                    if layer_idx == 1:
                        t2 = sb.tile([cBLK, cD], f32, tag="t2")
                        nc.vector.tensor_tensor(out=t2[:], in0=t1[:],
                                                in1=bb1[:cBLK, :], op=Alu.add)
                        # relu(x)*m == relu(x*m) since m >= 0
                        t3 = sb.tile([cBLK, cD], f32, tag="t3")
                        m2v = m2res[:].rearrange("n (b d) -> n b d", b=cNB)
                        nc.vector.tensor_tensor(out=t3[:], in0=t2[:],
                                                in1=m2v[:, b, :], op=Alu.mult)
                        h1v = h1res[:].rearrange("n (b d) -> n b d", b=cNB)
                        nc.vector.tensor_scalar(
                            out=h1v[:, b, :], in0=t3[:], scalar1=0.0,
                            scalar2=None, op0=Alu.max)
                    else:
                        h2v = h2res[:].rearrange("n (b d) -> n b d", b=cNB)
                        nc.vector.tensor_tensor(out=h2v[:, b, :], in0=t1[:],
                                                in1=bb2[:cBLK, :], op=Alu.add)
                        # transpose h2 block for the decoder FC
                        pt = pst.tile([cD, cBLK], mybir.dt.float32, tag="pt")
                        nc.tensor.transpose(pt[:], h2v[:, b, :],
                                            ident[:cBLK, :cBLK])
                        hT = sb.tile([cD, cBLK], bf16, tag="hT")
                        nc.vector.tensor_copy(out=hT[:], in_=pt[:])
                        pz = psz.tile([cBLK, cDEC], mybir.dt.float32,
                                      tag="pz")
                        nc.tensor.matmul(pz[:], lhsT=hT[:], rhs=wd[:],
                                         start=True, stop=True)
                        zsb = sb.tile([cBLK, cDEC], f32, tag="zsb")
                        nc.vector.tensor_tensor(out=zsb[:], in0=pz[:],
                                                in1=bbd[:cBLK, :], op=Alu.add)
                        zv = zres[:].rearrange("n (b d) -> n b d", b=cNB)
                        nc.vector.tensor_copy(out=zv[:, b, :], in_=zsb[:])
                        # log-softmax over DEC
                        negm = sb.tile([cBLK, 1], f32, tag="negm")
                        nc.vector.tensor_reduce(
                            out=negm[:], in_=zsb[:],
                            axis=mybir.AxisListType.X, op=Alu.max,
                            negate=True)
                        esb = sb.tile([cBLK, cDEC], f32, tag="esb")
                        nc.scalar.activation(esb[:], zsb[:], Act.Exp,
                                             bias=negm[:, 0:1], scale=1.0)
                        ssum = sb.tile([cBLK, 1], f32, tag="ssum")
                        nc.vector.tensor_reduce(
                            out=ssum[:], in_=esb[:],
                            axis=mybir.AxisListType.X, op=Alu.add)
                        lns = sb.tile([cBLK, 1], f32, tag="lns")
                        nc.scalar.activation(lns[:], ssum[:], Act.Ln)
                        mpl = sb.tile([cBLK, 1], f32, tag="mpl")
                        nc.vector.tensor_tensor(out=mpl[:], in0=lns[:],
                                                in1=negm[:], op=Alu.subtract)
                        lsv = lsres[:].rearrange("n (b d) -> n b d", b=cNB)
                        nc.vector.tensor_scalar(
                            out=lsv[:, b, :], in0=zsb[:], scalar1=mpl[:, 0:1],
                            scalar2=None, op0=Alu.subtract)

            # ---- layer 1 ----
            emit_layer(h0f, w1, 1)
            nc.sync.dma_start(
                out=h1b[:].rearrange("(b n) d -> n b d", n=cBLK),
                in_=h1res[:].rearrange("n (b d) -> n b d", b=cNB))
            if debug:
                nc.gpsimd.dma_start(
                    out=dbg_h1.rearrange("(b n) d -> n b d", n=cBLK),
                    in_=h1res[:].rearrange("n (b d) -> n b d", b=cNB))
            nc.gpsimd.collective_compute(
                "AllGather", Alu.bypass, replica_groups=AG,
                ins=[h1b[:].opt()], outs=[h1f[:].opt()])

            # ---- layer 2 + decoder per-block ----
            emit_layer(h1f, w2, 2)
            h2bf = big.tile([cBLK, cNB * cD], bf16, tag="bigC")
            nc.vector.tensor_copy(out=h2bf[:], in_=h2res[:])
            nc.sync.dma_start(
                out=h2_out.rearrange("(b n) d -> n b d", n=cBLK),
                in_=h2bf[:].rearrange("n (b d) -> n b d", b=cNB))
            nc.sync.dma_start(
                out=zb[:].rearrange("(b n) d -> n b d", n=cBLK),
                in_=zres[:].rearrange("n (b d) -> n b d", b=cNB))
            nc.gpsimd.collective_compute(
                "AllGather", Alu.bypass, replica_groups=AG,
                ins=[zb[:].opt()], outs=[zf[:].opt()])
            if debug:
                nc.gpsimd.dma_start(
                    out=dbg_z.rearrange("(b n) d -> n b d", n=cBLK),
                    in_=zres[:].rearrange("n (b d) -> n b d", b=cNB))
                nc.sync.dma_start(
                    out=dbg_ls.rearrange("(b n) d -> n b d", n=cBLK),
                    in_=lsres[:].rearrange("n (b d) -> n b d", b=cNB))
            nc.sync.dma_start(
                out=lsd[0:cNP, :].rearrange("(b n) d -> n b d", n=cBLK),
                in_=lsres[:].rearrange("n (b d) -> n b d", b=cNB))
            pad_rows = NTI * cP - cNP
            if pad_rows:
                zpad = sb.tile([pad_rows, cDEC], f32, tag="zpad")
                nc.vector.memset(zpad[:], 0.0)
                nc.sync.dma_start(out=lsd[cNP:, :], in_=zpad[:])

            # ---- decoder tail ----
            lst = big.tile([cP, NTI * cDEC], f32, tag="bigA")
            nc.sync.dma_start(
                out=lst[:].rearrange("p (i d) -> p i d", i=NTI),
                in_=lsd[:].rearrange("(i p) d -> p i d", p=cP))
            zg = big.tile([cP, NTI * cDEC], bf16, tag="bigB")
            zgv = zg[:].rearrange("p (i d) -> p i d", i=NTI)
            for i in range(NTI):
                nc.gpsimd.indirect_dma_start(
                    out=zgv[:, i, :],
                    out_offset=None,
                    in_=zf[:],
                    in_offset=bass.IndirectOffsetOnAxis(
                        ap=shufs[:, i:i + 1], axis=0),
                )
            pr = big.tile([cP, NTI * cDEC], f32, tag="big20")
            nc.vector.tensor_tensor(out=pr[:], in0=zg[:], in1=lst[:],
                                    op=Alu.mult)
            ac = sb.tile([cP, 1], f32, tag="ac")
            nc.vector.tensor_reduce(
                out=ac[:], in_=pr[:].rearrange("p (i d) -> p i d", i=NTI),
                axis=mybir.AxisListType.XY, op=Alu.add)
            ones = cst.tile([cP, 1], f32)
            nc.vector.memset(ones[:], 1.0)
            pd = psz.tile([1, 1], mybir.dt.float32, tag="pd")
            nc.tensor.matmul(pd[:], lhsT=ac[:], rhs=ones[:], start=True,
                             stop=True)
            dsb = sb.tile([1, 1], f32, tag="dsb")
            nc.vector.tensor_copy(out=dsb[:], in_=pd[:])
            nc.sync.dma_start(out=dec_out[:, :], in_=dsb[:])

    nc.compile()
    return nc


# ---- host-side sharding / preprocessing --------------------------------------

def host_prep(cfg, x, src, dst, shuffled_index, W1, b1, W2, b2, Wd, bd,
              mask1, mask2):
    """Partition the graph and pack per-core DMA-friendly arrays.
    Returns (in_maps, T, NTI)."""
    cN, cD, cDEC, cNC, cNP, cBLK, cNB, cP = (
        cfg["N"], cfg["D"], cfg["DEC"], cfg["NC"], cfg["NP"], cfg["BLK"],
        cfg["NB"], cfg["P"])
    bf = ml_dtypes.bfloat16

    src = np.asarray(src).astype(np.int64)
    dst = np.asarray(dst).astype(np.int64)
    shuffled_index = np.asarray(shuffled_index).astype(np.int64)
    x = np.ascontiguousarray(np.asarray(x, dtype=np.float32))

    deg = np.bincount(dst, minlength=cN)
    scale = (1.0 / (deg + 1.0)).astype(np.float32)

    # sort edges by dst; blocks never straddle cores since NP % BLK == 0
    order = np.argsort(dst, kind="stable")
    s_src = src[order]
    s_dst = dst[order]
    gblk = s_dst // cBLK                      # global block id, 0..NC*NB-1
    nblk = cNC * cNB
    counts = np.bincount(gblk, minlength=nblk)
    ptr = np.zeros(nblk + 1, np.int64)
    np.cumsum(counts, out=ptr[1:])
    tot = counts + cBLK                       # + self edges
    T = int(np.ceil(tot.max() / cP))
    slots = T * cP

    src_all = np.zeros(nblk * slots, np.int32)          # pad -> row 0
    dst_all = np.full(nblk * slots, -1, np.int8)        # pad -> no slot
    # real edges
    rank = np.arange(len(s_src)) - ptr[gblk]
    pos = gblk * slots + rank
    src_all[pos] = s_src
    dst_all[pos] = (s_dst % cBLK).astype(np.int8)
    # self edges
    g = np.arange(nblk)
    vbase = (g // cNB) * cNP + (g % cNB) * cBLK
    selfpos = (g[:, None] * slots + counts[:, None]
               + np.arange(cBLK)[None, :]).ravel()
    src_all[selfpos] = (vbase[:, None] + np.arange(cBLK)[None, :]).ravel()
    dst_all[selfpos] = np.tile(np.arange(cBLK, dtype=np.int8), nblk)

    # [core, (b t p)] -> [core][P, NB*T] with col = b*T+t, lane p
    src_all = src_all.reshape(cNC, cNB * T, cP)
    dst_all = dst_all.reshape(cNC, cNB * T, cP)

    NTI = -(-cNP // cP)
    shuf_pad = np.zeros(cNC * NTI * cP, np.int32)
    shuf_pad[: cNC * cNP] = shuffled_index.reshape(cNC, cNP).ravel()
    # careful: padding must be per core
    shuf_pc = np.zeros((cNC, NTI * cP), np.int32)
    shuf_pc[:, :cNP] = shuffled_index.reshape(cNC, cNP)
    shuf_pc = shuf_pc.reshape(cNC, NTI, cP)

    scale_pc = scale.reshape(cNC, cNB, cBLK)

    w1t = np.ascontiguousarray(np.asarray(W1, np.float32).T).astype(bf)
    w2t = np.ascontiguousarray(np.asarray(W2, np.float32).T).astype(bf)
    wdt = np.ascontiguousarray(np.asarray(Wd, np.float32).T).astype(bf)
    b1bc = np.ascontiguousarray(
        np.broadcast_to(np.asarray(b1, np.float32), (cP, cD)))
    b2bc = np.ascontiguousarray(
        np.broadcast_to(np.asarray(b2, np.float32), (cP, cD)))
    bdbc = np.ascontiguousarray(
        np.broadcast_to(np.asarray(bd, np.float32), (cP, cDEC)))
    iota = np.ascontiguousarray(
        np.broadcast_to(np.arange(cBLK, dtype=np.float32), (cP, cBLK)))

    m1 = np.asarray(mask1, np.float32).astype(np.int8).reshape(cNC, cNP, cD)
    xbf = x.astype(bf)
    m2 = np.asarray(mask2, np.float32).astype(np.int8).reshape(cNC, cNP, cD)
    xs = xbf.reshape(cNC, cNP, cD)

    in_maps = []
    for c in range(cNC):
        in_maps.append({
            "x_own": np.ascontiguousarray(xs[c]),
            "m1_own": np.ascontiguousarray(m1[c]),
            "m2_own": np.ascontiguousarray(m2[c]),
            "src_e": np.ascontiguousarray(src_all[c].T),
            "dst_e": np.ascontiguousarray(dst_all[c].T),
            "scl_own": np.ascontiguousarray(scale_pc[c].T),
            "shuf": np.ascontiguousarray(
                shuf_pc[c].T.reshape(cP, NTI)),
            "w1t": w1t, "w2t": w2t, "wdt": wdt,
            "b1bc": b1bc, "b2bc": b2bc, "bdbc": bdbc,
            "iota": iota,
        })
    return in_maps, T, NTI


# ---- execution ---------------------------------------------------------------

class PjrtRunner:
    """Mirror of bass2jax.run_bass_via_pjrt that keeps the jitted callable
    so repeat executions skip retracing."""

    def __init__(self, nc, n_cores):
        import jax
        import numpy as np
        from jax.sharding import Mesh, PartitionSpec
        from jax.experimental.shard_map import shard_map
        import concourse.mybir as mybir
        from concourse import bass2jax

        bass2jax.install_neuronx_cc_hook()
        self.jax = jax
        self.nc = nc
        self.n_cores = n_cores
        partition_name = (nc.partition_id_tensor.name
                          if nc.partition_id_tensor else None)
        in_names, out_names, out_avals, zero_outs = [], [], [], []
        for alloc in nc.m.functions[0].allocations:
            if not isinstance(alloc, mybir.MemoryLocationSet):
                continue
            name = alloc.memorylocations[0].name
            if alloc.kind == "ExternalInput":
                if name != partition_name:
                    in_names.append(name)
            elif alloc.kind == "ExternalOutput":
                shape = tuple(alloc.tensor_shape)
                dtype = mybir.dt.np(alloc.dtype)
                out_names.append(name)
                out_avals.append(jax.core.ShapedArray(shape, dtype))
                zero_outs.append(np.zeros(shape, dtype))
        self.in_names = in_names
        self.out_names = out_names
        self.out_avals = out_avals
        self.zero_outs = zero_outs
        n_params = len(in_names)
        n_outs = len(out_avals)
        all_names = list(in_names) + list(out_names)
        if partition_name is not None:
            all_names.append(partition_name)

        def _body(*args):
            operands = list(args)
            if partition_name is not None:
                operands.append(bass2jax.partition_id_tensor())
            outs = bass2jax._bass_exec_p.bind(
                *operands, out_avals=tuple(out_avals),
                in_names=tuple(all_names), out_names=tuple(out_names),
                lowering_input_output_aliases=(),
                sim_require_finite=True, sim_require_nnan=True, nc=nc)
            return tuple(outs)

        devices = jax.devices()[:n_cores]
        mesh = Mesh(np.asarray(devices), ("core",))
        in_specs = (PartitionSpec("core"),) * (n_params + n_outs)
        out_specs = (PartitionSpec("core"),) * n_outs
        self.sharded = jax.jit(
            shard_map(_body, mesh=mesh, in_specs=in_specs,
                      out_specs=out_specs, check_rep=False),
            keep_unused=True)
        self._dev_args = None

    def put(self, in_maps):
        np_ = np
        concat_in = [
            np_.concatenate([np_.asarray(in_maps[c][nm])
                             for c in range(self.n_cores)], axis=0)
            for nm in self.in_names]
        concat_zeros = [
            np_.zeros((self.n_cores * z.shape[0], *z.shape[1:]), z.dtype)
            for z in self.zero_outs]
        self._dev_args = [self.jax.device_put(a)
                          for a in concat_in + concat_zeros]

    def run(self):
        outs = self.sharded(*self._dev_args)
        self.jax.block_until_ready(outs)
        return [
            {nm: np.asarray(outs[i]).reshape(
                self.n_cores, *self.out_avals[i].shape)[c]
             for i, nm in enumerate(self.out_names)}
            for c in range(self.n_cores)]


def _compute_masks(shape1, shape2):
    import jax
    cpu = jax.devices("cpu")[0]
    with jax.default_device(cpu):
        dk = jax.random.split(jax.random.key(42), 2)
        m1 = np.asarray(
            jax.random.bernoulli(dk[0], 0.5, shape1)).astype(np.float32) * 2.0
        m2 = np.asarray(
            jax.random.bernoulli(dk[1], 0.5, shape2)).astype(np.float32) * 2.0
    return m1, m2


def kernel(x, src, dst, shuffled_index, W1, b1, W2, b2, Wd, bd):
    cfg = _FULL_CFG
    mask1, mask2 = _compute_masks((cfg["N"], cfg["D"]), (cfg["N"], cfg["D"]))
    in_maps, T, NTI = host_prep(cfg, x, src, dst, shuffled_index,
                                W1, b1, W2, b2, Wd, bd, mask1, mask2)
    key = (T, NTI)
    if key not in _program_cache:
        nc = build_program(cfg, T, NTI)
        _program_cache[key] = PjrtRunner(nc, cfg["NC"])
    runner = _program_cache[key]
    runner.put(in_maps)
    res = runner.run()
    h = np.concatenate([np.asarray(res[c]["h2_out"]).astype(np.float32)
                        for c in range(cfg["NC"])], axis=0)
    dec = -sum(float(res[c]["dec_out"][0, 0]) for c in range(cfg["NC"])) \
        / cfg["N"]
    return h.astype(np.float32), np.float32(dec)


# revision 13
# speedup vs baseline: 1.3706x; 1.0032x over previous
"""Trainium2 Bass kernel for nn_Encoder_14018773254741 (2-layer SAGE-GCN
encoder + soft-target CE decoder) on 8 NeuronCores.

Strategy (1D node partition):
  - 40000 nodes split 5000/core; each core owns the edges whose dst lands in
    its range (plus one explicit self-edge per node, which implements the
    "+ h_v" term of the gcn aggregator).
  - Edges are sorted by dst and packed into fixed-size tiles of 128 on the
    host; per 125-node block the aggregation runs as: batched indirect-DMA
    gather of h[src] rows (bf16 table) -> one-hot selection matrix built
    on-chip (iota == dst_local) -> PE matmuls accumulating neigh^T in PSUM.
  - The 128-dim linears replicate weights; deg-scaling is folded in after
    the FC ((S*A) @ W == S*(A @ W)); dropout masks are precomputed constants
    (fixed jax key) applied on-chip.
  - Halo exchange: the full (quantized bf16) node-feature table is
    AllGather-ed between layers; decoder logits z are AllGather-ed once for
    the shuffled-row gather.

Outputs: h [40000,128] f32 (node-major shards concatenated on host) and the
scalar dec (per-core partial sums, final mean on host).
"""

import numpy as np
import ml_dtypes

# ---- hardcoded problem config ------------------------------------------------
N = 40000          # nodes
D = 128            # feature dim (in = hid = out)
DEC = 64           # decoder dim
NC = 8             # cores
NP = N // NC       # nodes per core
BLK = 125          # dst nodes per aggregation block
NB = NP // BLK     # blocks per core
P = 128            # partitions / edge-tile size

_FULL_CFG = dict(N=N, D=D, DEC=DEC, NC=NC, NP=NP, BLK=BLK, NB=NB, P=P)

_program_cache = {}


# ---- device program ----------------------------------------------------------

def build_program(cfg, T, NTI, debug=False):
    """Build the 8-core SPMD Bass program. T = edge tiles per block,
    NTI = decoder gather tiles (= ceil(NP/128))."""
    import concourse.bass as bass
    import concourse.bacc as bacc
    import concourse.mybir as mybir
    import concourse.tile as tile
    from concourse.masks import make_identity

    f32 = mybir.dt.float32
    bf16 = mybir.dt.bfloat16
    i32 = mybir.dt.int32
    Alu = mybir.AluOpType
    Act = mybir.ActivationFunctionType

    cN, cD, cDEC, cNC, cNP, cBLK, cNB, cP = (
        cfg["N"], cfg["D"], cfg["DEC"], cfg["NC"], cfg["NP"], cfg["BLK"],
        cfg["NB"], cfg["P"])
    AG = [list(range(cNC))]

    nc = bacc.Bacc("TRN2", target_bir_lowering=False, debug=False,
                   num_devices=cNC)

    def din(name, shape, dt):
        return nc.dram_tensor(name, shape, dt, kind="ExternalInput").ap()

    x_own = din("x_own", [cNP, cD], bf16)
    m1_own = din("m1_own", [cNP, cD], mybir.dt.int8)
    m2_own = din("m2_own", [cNP, cD], mybir.dt.int8)
    src_e = din("src_e", [cP, cNB * T], i32)
    src2_e = din("src2_e", [cP, cNB * T], i32)
    dst_e = din("dst_e", [cP, cNB * T], mybir.dt.int8)
    scl_own = din("scl_own", [cBLK, cNB], f32)
    shuf = din("shuf", [cP, NTI], i32)
    w1t = din("w1t", [cD, cD], bf16)
    w2t = din("w2t", [cD, cD], bf16)
    wdt = din("wdt", [cD, cDEC], bf16)
    b1bc = din("b1bc", [cP, cD], f32)
    b2bc = din("b2bc", [cP, cD], f32)
    bdbc = din("bdbc", [cP, cDEC], f32)
    iota_in = din("iota", [cP, cBLK], f32)

    h2_out = nc.dram_tensor("h2_out", [cNP, cD], bf16,
                            kind="ExternalOutput").ap()
    dec_out = nc.dram_tensor("dec_out", [1, 1], f32,
                             kind="ExternalOutput").ap()
    if debug:
        dbg_h0 = nc.dram_tensor("dbg_h0", [cNP, cD], f32,
                                kind="ExternalOutput").ap()
        dbg_msg = nc.dram_tensor("dbg_msg", [cP, T * cD], f32,
                                 kind="ExternalOutput").ap()
        dbg_mp = nc.dram_tensor("dbg_mp", [cP, T * cBLK], f32,
                                kind="ExternalOutput").ap()
        dbg_nT = nc.dram_tensor("dbg_nT", [cD, cBLK], f32,
                                kind="ExternalOutput").ap()
        dbg_h1 = nc.dram_tensor("dbg_h1", [cNP, cD], f32,
                                kind="ExternalOutput").ap()
        dbg_z = nc.dram_tensor("dbg_z", [cNP, cDEC], f32,
                               kind="ExternalOutput").ap()
        dbg_ls = nc.dram_tensor("dbg_ls", [cNP, cDEC], f32,
                                kind="ExternalOutput").ap()

    with tile.TileContext(nc) as tc:
        with (
            tc.tile_pool(name="cst", bufs=1) as cst,
            tc.tile_pool(name="big", bufs=1) as big,
            tc.tile_pool(name="sb", bufs=2) as sb,
            tc.tile_pool(name="psn", bufs=2, space="PSUM") as psn,
            tc.tile_pool(name="psf", bufs=2, space="PSUM") as psf,
            tc.tile_pool(name="pst", bufs=1, space="PSUM") as pst,
            tc.tile_pool(name="psz", bufs=1, space="PSUM") as psz,
            tc.tile_pool(name="dram", bufs=1, space="DRAM") as dram,
        ):
            # ---- persistent constants ----
            srcs = cst.tile([cP, cNB * T], i32)
            nc.sync.dma_start(out=srcs[:], in_=src_e[:, :])
            srcs2 = cst.tile([cP, cNB * T], i32)
            nc.sync.dma_start(out=srcs2[:], in_=src2_e[:, :])
            dsts = cst.tile([cP, cNB * T], mybir.dt.int8)
            nc.sync.dma_start(out=dsts[:], in_=dst_e[:, :])
            iot = cst.tile([cP, cBLK], f32)
            nc.sync.dma_start(out=iot[:], in_=iota_in[:, :])
            sclc = cst.tile([cBLK, cNB], f32)
            nc.sync.dma_start(out=sclc[:], in_=scl_own[:, :])
            w1 = cst.tile([cD, cD], bf16)
            nc.sync.dma_start(out=w1[:], in_=w1t[:, :])
            w2 = cst.tile([cD, cD], bf16)
            nc.sync.dma_start(out=w2[:], in_=w2t[:, :])
            wd = cst.tile([cD, cDEC], bf16)
            nc.sync.dma_start(out=wd[:], in_=wdt[:, :])
            bb1 = cst.tile([cP, cD], f32)
            nc.sync.dma_start(out=bb1[:], in_=b1bc[:, :])
            bb2 = cst.tile([cP, cD], f32)
            nc.sync.dma_start(out=bb2[:], in_=b2bc[:, :])
            bbd = cst.tile([cP, cDEC], f32)
            nc.sync.dma_start(out=bbd[:], in_=bdbc[:, :])
            shufs = cst.tile([cP, NTI], i32)
            nc.sync.dma_start(out=shufs[:], in_=shuf[:, :])
            ident = cst.tile([cP, cP], f32)
            make_identity(nc, ident[:])
            m2res = cst.tile([cBLK, cNB * cD], mybir.dt.int8)
            nc.sync.dma_start(
                out=m2res[:].rearrange("n (b d) -> n b d", b=cNB),
                in_=m2_own.rearrange("(b n) d -> n b d", n=cBLK))

            # persistent per-layer result accumulators
            h1res = cst.tile([cBLK, cNB * cD], bf16)
            h2res = cst.tile([cBLK, cNB * cD], f32)
            zres = cst.tile([cBLK, cNB * cDEC], bf16)
            lsres = cst.tile([cBLK, cNB * cDEC], f32)

            # ---- DRAM intermediates ----
            h0b = dram.tile([cNP, cD], bf16)
            h0f = dram.tile([cN, cD], bf16)
            NCH = 4 if cNB % 4 == 0 and cNB >= 4 else 1
            CHB = cNB // NCH            # blocks per chunk
            CHR = CHB * cBLK            # rows per chunk per core
            h1b = dram.tile([cNP, cD], bf16)
            h1f = dram.tile([cN, cD], bf16)
            zb = dram.tile([cNP, cDEC], bf16)
            zf = dram.tile([cN, cDEC], bf16)
            lsd = dram.tile([NTI * cP, cDEC], f32)

            # ---- phase A: h0 = x * mask1 (bf16 table) ----
            xres = big.tile([cBLK, cNB * cD], bf16, tag="big20")
            nc.sync.dma_start(
                out=xres[:].rearrange("n (b d) -> n b d", b=cNB),
                in_=x_own.rearrange("(b n) d -> n b d", n=cBLK))
            m1res = big.tile([cBLK, cNB * cD], mybir.dt.int8, tag="bigA")
            nc.sync.dma_start(
                out=m1res[:].rearrange("n (b d) -> n b d", b=cNB),
                in_=m1_own.rearrange("(b n) d -> n b d", n=cBLK))
            h0res = big.tile([cBLK, cNB * cD], bf16, tag="bigB")
            nc.vector.tensor_tensor(out=h0res[:], in0=xres[:], in1=m1res[:],
                                    op=Alu.mult)
            nc.sync.dma_start(
                out=h0b[:].rearrange("(b n) d -> n b d", n=cBLK),
                in_=h0res[:].rearrange("n (b d) -> n b d", b=cNB))
            if debug:
                nc.gpsimd.dma_start(
                    out=dbg_h0.rearrange("(b n) d -> n b d", n=cBLK),
                    in_=h0res[:].rearrange("n (b d) -> n b d", b=cNB))
            nc.gpsimd.collective_compute(
                "AllGather", Alu.bypass, replica_groups=AG,
                ins=[h0b[:].opt()], outs=[h0f[:].opt()])

            # ---- shared aggregation + FC ----
            def emit_layer(table, wt, layer_idx, block_done=None):
                src_tbl = srcs if layer_idx == 1 else srcs2
                for b in range(cNB):
                    # per-tile [128,1] gathers of this block's edge sources
                    # (the only indirect-DMA form the HW handles correctly)
                    msg = sb.tile([cP, T * cD], bf16, tag="msg")
                    msgv0 = msg[:].rearrange("p (t d) -> p t d", t=T)
                    for t in range(T):
                        nc.gpsimd.indirect_dma_start(
                            out=msgv0[:, t, :],
                            out_offset=None,
                            in_=table[:],
                            in_offset=bass.IndirectOffsetOnAxis(
                                ap=src_tbl[:, b * T + t:b * T + t + 1],
                                axis=0),
                        )
                    # selection matrix M[e, t, s] = (dst_local[e,t] == s)
                    mp = sb.tile([cP, T * cBLK], bf16, tag="mp")
                    dstb = dsts[:, b * T:(b + 1) * T].to_broadcast(
                        [cP, T, cBLK])
                    iob = iot[:].rearrange("p (o k) -> p o k", o=1)\
                        .to_broadcast([cP, T, cBLK])
                    nc.vector.tensor_tensor(
                        out=mp[:].rearrange("p (t k) -> p t k", t=T),
                        in0=dstb, in1=iob, op=Alu.is_equal)
                    # neigh^T[f, s] accumulated over edge tiles
                    pn = psn.tile([cD, cBLK], mybir.dt.float32, tag="pn")
                    msgv = msg[:].rearrange("p (t d) -> p t d", t=T)
                    mpv = mp[:].rearrange("p (t k) -> p t k", t=T)
                    for t in range(T):
                        nc.tensor.matmul(
                            pn[:], lhsT=msgv[:, t, :], rhs=mpv[:, t, :],
                            start=(t == 0), stop=(t == T - 1))
                    nT = sb.tile([cD, cBLK], bf16, tag="nT")
                    nc.vector.tensor_copy(out=nT[:], in_=pn[:])
                    if debug and layer_idx == 1 and b == 0:
                        nc.gpsimd.dma_start(out=dbg_msg[:, :], in_=msg[:])
                        nc.gpsimd.dma_start(out=dbg_mp[:, :], in_=mp[:])
                        nc.gpsimd.dma_start(out=dbg_nT[:, :], in_=nT[:])
                    # FC: out[n, o] = neigh^T.T @ W^T  (scale folded after)
                    pf = psf.tile([cBLK, cD], mybir.dt.float32, tag="pf")
                    nc.tensor.matmul(pf[:], lhsT=nT[:], rhs=wt[:],
                                     start=True, stop=True)
                    t1 = sb.tile([cBLK, cD], f32, tag="t1")
                    nc.vector.tensor_scalar(
                        out=t1[:], in0=pf[:], scalar1=sclc[:, b:b + 1],
                        scalar2=None, op0=Alu.mult)
                    if layer_idx == 1:
                        t2 = sb.tile([cBLK, cD], f32, tag="t2")
                        nc.vector.tensor_tensor(out=t2[:], in0=t1[:],
                                                in1=bb1[:cBLK, :], op=Alu.add)
                        # relu(x)*m == relu(x*m) since m >= 0
                        t3 = sb.tile([cBLK, cD], f32, tag="t3")
                        m2v = m2res[:].rearrange("n (b d) -> n b d", b=cNB)
                        nc.vector.tensor_tensor(out=t3[:], in0=t2[:],
                                                in1=m2v[:, b, :], op=Alu.mult)
                        h1v = h1res[:].rearrange("n (b d) -> n b d", b=cNB)
                        nc.vector.tensor_scalar(
                            out=h1v[:, b, :], in0=t3[:], scalar1=0.0,
                            scalar2=None, op0=Alu.max)
                        if block_done is not None:
                            block_done(b)
                    else:
                        h2v = h2res[:].rearrange("n (b d) -> n b d", b=cNB)
                        nc.vector.tensor_tensor(out=h2v[:, b, :], in0=t1[:],
                                                in1=bb2[:cBLK, :], op=Alu.add)
                        # transpose h2 block for the decoder FC
                        pt = pst.tile([cD, cBLK], mybir.dt.float32, tag="pt")
                        nc.tensor.transpose(pt[:], h2v[:, b, :],
                                            ident[:cBLK, :cBLK])
                        hT = sb.tile([cD, cBLK], bf16, tag="hT")
                        nc.vector.tensor_copy(out=hT[:], in_=pt[:])
                        pz = psz.tile([cBLK, cDEC], mybir.dt.float32,
                                      tag="pz")
                        nc.tensor.matmul(pz[:], lhsT=hT[:], rhs=wd[:],
                                         start=True, stop=True)
                        zsb = sb.tile([cBLK, cDEC], f32, tag="zsb")
                        nc.vector.tensor_tensor(out=zsb[:], in0=pz[:],
                                                in1=bbd[:cBLK, :], op=Alu.add)
                        zv = zres[:].rearrange("n (b d) -> n b d", b=cNB)
                        nc.vector.tensor_copy(out=zv[:, b, :], in_=zsb[:])
                        # log-softmax over DEC
                        negm = sb.tile([cBLK, 1], f32, tag="negm")
                        nc.vector.tensor_reduce(
                            out=negm[:], in_=zsb[:],
                            axis=mybir.AxisListType.X, op=Alu.max,
                            negate=True)
                        esb = sb.tile([cBLK, cDEC], f32, tag="esb")
                        nc.scalar.activation(esb[:], zsb[:], Act.Exp,
                                             bias=negm[:, 0:1], scale=1.0)
                        ssum = sb.tile([cBLK, 1], f32, tag="ssum")
                        nc.vector.tensor_reduce(
                            out=ssum[:], in_=esb[:],
                            axis=mybir.AxisListType.X, op=Alu.add)
                        lns = sb.tile([cBLK, 1], f32, tag="lns")
                        nc.scalar.activation(lns[:], ssum[:], Act.Ln)
                        mpl = sb.tile([cBLK, 1], f32, tag="mpl")
                        nc.vector.tensor_tensor(out=mpl[:], in0=lns[:],
                                                in1=negm[:], op=Alu.subtract)
                        lsv = lsres[:].rearrange("n (b d) -> n b d", b=cNB)
                        nc.vector.tensor_scalar(
                            out=lsv[:, b, :], in0=zsb[:], scalar1=mpl[:, 0:1],
                            scalar2=None, op0=Alu.subtract)

            # ---- layer 1 (h1 AllGather chunked to overlap compute) ----
            def h1_block_done(b):
                if (b + 1) % CHB != 0:
                    return
                k = b // CHB
                bs = k * CHB
                nc.sync.dma_start(
                    out=h1b[k * CHR:(k + 1) * CHR, :].rearrange(
                        "(b n) d -> n b d", n=cBLK),
                    in_=h1res[:].rearrange("n (b d) -> n b d", b=cNB)
                    [:, bs:bs + CHB, :])
                nc.gpsimd.collective_compute(
                    "AllGather", Alu.bypass, replica_groups=AG,
                    ins=[h1b[k * CHR:(k + 1) * CHR, :].opt()],
                    outs=[h1f[k * cNC * CHR:(k + 1) * cNC * CHR, :].opt()])

            emit_layer(h0f, w1, 1, block_done=h1_block_done)
            if debug:
                nc.gpsimd.dma_start(
                    out=dbg_h1.rearrange("(b n) d -> n b d", n=cBLK),
                    in_=h1res[:].rearrange("n (b d) -> n b d", b=cNB))


            # ---- layer 2 + decoder per-block ----
            emit_layer(h1f, w2, 2)
            h2bf = big.tile([cBLK, cNB * cD], bf16, tag="bigC")
            nc.vector.tensor_copy(out=h2bf[:], in_=h2res[:])
            nc.sync.dma_start(
                out=h2_out.rearrange("(b n) d -> n b d", n=cBLK),
                in_=h2bf[:].rearrange("n (b d) -> n b d", b=cNB))
            nc.sync.dma_start(
                out=zb[:].rearrange("(b n) d -> n b d", n=cBLK),
                in_=zres[:].rearrange("n (b d) -> n b d", b=cNB))
            nc.gpsimd.collective_compute(
                "AllGather", Alu.bypass, replica_groups=AG,
                ins=[zb[:].opt()], outs=[zf[:].opt()])
            if debug:
                nc.gpsimd.dma_start(
                    out=dbg_z.rearrange("(b n) d -> n b d", n=cBLK),
                    in_=zres[:].rearrange("n (b d) -> n b d", b=cNB))
                nc.sync.dma_start(
                    out=dbg_ls.rearrange("(b n) d -> n b d", n=cBLK),
                    in_=lsres[:].rearrange("n (b d) -> n b d", b=cNB))
            nc.sync.dma_start(
                out=lsd[0:cNP, :].rearrange("(b n) d -> n b d", n=cBLK),
                in_=lsres[:].rearrange("n (b d) -> n b d", b=cNB))
            pad_rows = NTI * cP - cNP
            if pad_rows:
                zpad = sb.tile([pad_rows, cDEC], f32, tag="zpad")
                nc.vector.memset(zpad[:], 0.0)
                nc.sync.dma_start(out=lsd[cNP:, :], in_=zpad[:])

            # ---- decoder tail ----
            lst = big.tile([cP, NTI * cDEC], f32, tag="bigA")
            nc.sync.dma_start(
                out=lst[:].rearrange("p (i d) -> p i d", i=NTI),
                in_=lsd[:].rearrange("(i p) d -> p i d", p=cP))
            zg = big.tile([cP, NTI * cDEC], bf16, tag="bigB")
            zgv = zg[:].rearrange("p (i d) -> p i d", i=NTI)
            for i in range(NTI):
                nc.gpsimd.indirect_dma_start(
                    out=zgv[:, i, :],
                    out_offset=None,
                    in_=zf[:],
                    in_offset=bass.IndirectOffsetOnAxis(
                        ap=shufs[:, i:i + 1], axis=0),
                )
            pr = big.tile([cP, NTI * cDEC], f32, tag="big20")
            nc.vector.tensor_tensor(out=pr[:], in0=zg[:], in1=lst[:],
                                    op=Alu.mult)
            ac = sb.tile([cP, 1], f32, tag="ac")
            nc.vector.tensor_reduce(
                out=ac[:], in_=pr[:].rearrange("p (i d) -> p i d", i=NTI),
                axis=mybir.AxisListType.XY, op=Alu.add)
            ones = cst.tile([cP, 1], f32)
            nc.vector.memset(ones[:], 1.0)
            pd = psz.tile([1, 1], mybir.dt.float32, tag="pd")
            nc.tensor.matmul(pd[:], lhsT=ac[:], rhs=ones[:], start=True,
                             stop=True)
            dsb = sb.tile([1, 1], f32, tag="dsb")
            nc.vector.tensor_copy(out=dsb[:], in_=pd[:])
            nc.sync.dma_start(out=dec_out[:, :], in_=dsb[:])

    nc.compile()
    return nc


# ---- host-side sharding / preprocessing --------------------------------------

def host_prep(cfg, x, src, dst, shuffled_index, W1, b1, W2, b2, Wd, bd,
              mask1, mask2):
    """Partition the graph and pack per-core DMA-friendly arrays.
    Returns (in_maps, T, NTI)."""
    cN, cD, cDEC, cNC, cNP, cBLK, cNB, cP = (
        cfg["N"], cfg["D"], cfg["DEC"], cfg["NC"], cfg["NP"], cfg["BLK"],
        cfg["NB"], cfg["P"])
    bf = ml_dtypes.bfloat16

    src = np.asarray(src).astype(np.int64)
    dst = np.asarray(dst).astype(np.int64)
    shuffled_index = np.asarray(shuffled_index).astype(np.int64)
    x = np.ascontiguousarray(np.asarray(x, dtype=np.float32))

    deg = np.bincount(dst, minlength=cN)
    scale = (1.0 / (deg + 1.0)).astype(np.float32)

    # sort edges by dst; blocks never straddle cores since NP % BLK == 0
    order = np.argsort(dst, kind="stable")
    s_src = src[order]
    s_dst = dst[order]
    gblk = s_dst // cBLK                      # global block id, 0..NC*NB-1
    nblk = cNC * cNB
    counts = np.bincount(gblk, minlength=nblk)
    ptr = np.zeros(nblk + 1, np.int64)
    np.cumsum(counts, out=ptr[1:])
    tot = counts + cBLK                       # + self edges
    T = int(np.ceil(tot.max() / cP))
    slots = T * cP

    src_all = np.zeros(nblk * slots, np.int32)          # pad -> row 0
    dst_all = np.full(nblk * slots, -1, np.int8)        # pad -> no slot
    # real edges
    rank = np.arange(len(s_src)) - ptr[gblk]
    pos = gblk * slots + rank
    src_all[pos] = s_src
    dst_all[pos] = (s_dst % cBLK).astype(np.int8)
    # self edges
    g = np.arange(nblk)
    vbase = (g // cNB) * cNP + (g % cNB) * cBLK
    selfpos = (g[:, None] * slots + counts[:, None]
               + np.arange(cBLK)[None, :]).ravel()
    src_all[selfpos] = (vbase[:, None] + np.arange(cBLK)[None, :]).ravel()
    dst_all[selfpos] = np.tile(np.arange(cBLK, dtype=np.int8), nblk)

    # [core, (b t p)] -> [core][P, NB*T] with col = b*T+t, lane p
    src_all = src_all.reshape(cNC, cNB * T, cP)
    dst_all = dst_all.reshape(cNC, cNB * T, cP)

    NTI = -(-cNP // cP)
    shuf_pad = np.zeros(cNC * NTI * cP, np.int32)
    shuf_pad[: cNC * cNP] = shuffled_index.reshape(cNC, cNP).ravel()
    # careful: padding must be per core
    shuf_pc = np.zeros((cNC, NTI * cP), np.int32)
    shuf_pc[:, :cNP] = shuffled_index.reshape(cNC, cNP)
    shuf_pc = shuf_pc.reshape(cNC, NTI, cP)

    scale_pc = scale.reshape(cNC, cNB, cBLK)

    w1t = np.ascontiguousarray(np.asarray(W1, np.float32).T).astype(bf)
    w2t = np.ascontiguousarray(np.asarray(W2, np.float32).T).astype(bf)
    wdt = np.ascontiguousarray(np.asarray(Wd, np.float32).T).astype(bf)
    b1bc = np.ascontiguousarray(
        np.broadcast_to(np.asarray(b1, np.float32), (cP, cD)))
    b2bc = np.ascontiguousarray(
        np.broadcast_to(np.asarray(b2, np.float32), (cP, cD)))
    bdbc = np.ascontiguousarray(
        np.broadcast_to(np.asarray(bd, np.float32), (cP, cDEC)))
    iota = np.ascontiguousarray(
        np.broadcast_to(np.arange(cBLK, dtype=np.float32), (cP, cBLK)))

    m1 = np.asarray(mask1, np.float32).astype(np.int8).reshape(cNC, cNP, cD)
    xbf = x.astype(bf)
    m2 = np.asarray(mask2, np.float32).astype(np.int8).reshape(cNC, cNP, cD)
    xs = xbf.reshape(cNC, cNP, cD)

    # layer-2 gathers read h1f in chunk-major layout:
    # node g = c*NP + b*BLK + n -> k*(NC*CHR) + c*CHR + (b%CHB)*BLK + n
    NCH = 4 if cNB % 4 == 0 and cNB >= 4 else 1
    CHB = cNB // NCH
    CHR = CHB * cBLK
    g = src_all.reshape(-1).astype(np.int64)
    gc = g // cNP
    gb = (g % cNP) // cBLK
    gn = g % cBLK
    src2_all = ((gb // CHB) * (cNC * CHR) + gc * CHR
                + (gb % CHB) * cBLK + gn).astype(np.int32)
    src2_all = src2_all.reshape(cNC, cNB * T, cP)

    in_maps = []
    for c in range(cNC):
        in_maps.append({
            "x_own": np.ascontiguousarray(xs[c]),
            "m1_own": np.ascontiguousarray(m1[c]),
            "m2_own": np.ascontiguousarray(m2[c]),
            "src_e": np.ascontiguousarray(src_all[c].T),
            "src2_e": np.ascontiguousarray(src2_all[c].T),
            "dst_e": np.ascontiguousarray(dst_all[c].T),
            "scl_own": np.ascontiguousarray(scale_pc[c].T),
            "shuf": np.ascontiguousarray(
                shuf_pc[c].T.reshape(cP, NTI)),
            "w1t": w1t, "w2t": w2t, "wdt": wdt,
            "b1bc": b1bc, "b2bc": b2bc, "bdbc": bdbc,
            "iota": iota,
        })
    return in_maps, T, NTI


# ---- execution ---------------------------------------------------------------

class PjrtRunner:
    """Mirror of bass2jax.run_bass_via_pjrt that keeps the jitted callable
    so repeat executions skip retracing."""

    def __init__(self, nc, n_cores):
        import jax
        import numpy as np
        from jax.sharding import Mesh, PartitionSpec
        from jax.experimental.shard_map import shard_map
        import concourse.mybir as mybir
        from concourse import bass2jax

        bass2jax.install_neuronx_cc_hook()
        self.jax = jax
        self.nc = nc
        self.n_cores = n_cores
        partition_name = (nc.partition_id_tensor.name
                          if nc.partition_id_tensor else None)
        in_names, out_names, out_avals, zero_outs = [], [], [], []
        for alloc in nc.m.functions[0].allocations:
            if not isinstance(alloc, mybir.MemoryLocationSet):
                continue
            name = alloc.memorylocations[0].name
            if alloc.kind == "ExternalInput":
                if name != partition_name:
                    in_names.append(name)
            elif alloc.kind == "ExternalOutput":
                shape = tuple(alloc.tensor_shape)
                dtype = mybir.dt.np(alloc.dtype)
                out_names.append(name)
                out_avals.append(jax.core.ShapedArray(shape, dtype))
                zero_outs.append(np.zeros(shape, dtype))
        self.in_names = in_names
        self.out_names = out_names
        self.out_avals = out_avals
        self.zero_outs = zero_outs
        n_params = len(in_names)
        n_outs = len(out_avals)
        all_names = list(in_names) + list(out_names)
        if partition_name is not None:
            all_names.append(partition_name)

        def _body(*args):
            operands = list(args)
            if partition_name is not None:
                operands.append(bass2jax.partition_id_tensor())
            outs = bass2jax._bass_exec_p.bind(
                *operands, out_avals=tuple(out_avals),
                in_names=tuple(all_names), out_names=tuple(out_names),
                lowering_input_output_aliases=(),
                sim_require_finite=True, sim_require_nnan=True, nc=nc)
            return tuple(outs)

        devices = jax.devices()[:n_cores]
        mesh = Mesh(np.asarray(devices), ("core",))
        in_specs = (PartitionSpec("core"),) * (n_params + n_outs)
        out_specs = (PartitionSpec("core"),) * n_outs
        self.sharded = jax.jit(
            shard_map(_body, mesh=mesh, in_specs=in_specs,
                      out_specs=out_specs, check_rep=False),
            keep_unused=True)
        self._dev_args = None

    def put(self, in_maps):
        np_ = np
        concat_in = [
            np_.concatenate([np_.asarray(in_maps[c][nm])
                             for c in range(self.n_cores)], axis=0)
            for nm in self.in_names]
        concat_zeros = [
            np_.zeros((self.n_cores * z.shape[0], *z.shape[1:]), z.dtype)
            for z in self.zero_outs]
        self._dev_args = [self.jax.device_put(a)
                          for a in concat_in + concat_zeros]

    def run(self):
        outs = self.sharded(*self._dev_args)
        self.jax.block_until_ready(outs)
        return [
            {nm: np.asarray(outs[i]).reshape(
                self.n_cores, *self.out_avals[i].shape)[c]
             for i, nm in enumerate(self.out_names)}
            for c in range(self.n_cores)]


def _compute_masks(shape1, shape2):
    import jax
    cpu = jax.devices("cpu")[0]
    with jax.default_device(cpu):
        dk = jax.random.split(jax.random.key(42), 2)
        m1 = np.asarray(
            jax.random.bernoulli(dk[0], 0.5, shape1)).astype(np.float32) * 2.0
        m2 = np.asarray(
            jax.random.bernoulli(dk[1], 0.5, shape2)).astype(np.float32) * 2.0
    return m1, m2


def kernel(x, src, dst, shuffled_index, W1, b1, W2, b2, Wd, bd):
    cfg = _FULL_CFG
    mask1, mask2 = _compute_masks((cfg["N"], cfg["D"]), (cfg["N"], cfg["D"]))
    in_maps, T, NTI = host_prep(cfg, x, src, dst, shuffled_index,
                                W1, b1, W2, b2, Wd, bd, mask1, mask2)
    key = (T, NTI)
    if key not in _program_cache:
        nc = build_program(cfg, T, NTI)
        _program_cache[key] = PjrtRunner(nc, cfg["NC"])
    runner = _program_cache[key]
    runner.put(in_maps)
    res = runner.run()
    h = np.concatenate([np.asarray(res[c]["h2_out"]).astype(np.float32)
                        for c in range(cfg["NC"])], axis=0)
    dec = -sum(float(res[c]["dec_out"][0, 0]) for c in range(cfg["NC"])) \
        / cfg["N"]
    return h.astype(np.float32), np.float32(dec)
